# revision 17
# baseline (speedup 1.0000x reference)
"""SAGAN-style attention (nn_Attention_24927990186686) on 8 TRN2 cores.

reference:
  f = Wf@x+bf  [B,64,N]   g = Wg@x+bg  [B,64,N]   h = Wh@x+bh  [B,128,N]
  s = g^T f    [B,N,N]    beta = softmax(s, -1)
  o[c,n] = sum_m beta[n,m] h[c,m];  out = gamma*o + x     (B=8, N=4096)

Sharding: data-parallel over batch, one batch per core, params replicated.

Per-core algorithm ("orientation B" — score tiles transposed so the softmax
contraction (m) lands on the partition axis, which is what the second matmul
contracts over; no giant transposes needed):
  fg  = [Wf^T|Wg^T]^T @ x + [bf;bg]       [128, 4096]  (f rows 0:64, g 64:128)
  hT_j = (x_tile_j)^T... via matmul(lhsT=x[:,128j:128j+128], rhs=Wh^T) + bh
  per n-block (512 cols):
    for j in 32:  t_j = f_j^T g_blk (PSUM) -> e_j = exp(t_j) (ACT)
                  d_acc += e_j (DVE);  psum_o += hT_j^T e_j (PE, accum)
    d = ones^T d_acc (PE row-reduce) -> recip*gamma (DVE)
    bcast via ones outer-product (PE) -> out = psum_o*bcast + x (DVE)
softmax max-subtraction is skipped: |s| <~ 50 for these input distributions,
exp stays comfortably inside fp32 range, and normalization cancels the shift.
"""

import json
import sys
import types

if "/opt/trn_rl_repo" not in sys.path:
    sys.path.insert(0, "/opt/trn_rl_repo")

import numpy as np

import concourse.bass as bass
import concourse.tile as tile
from concourse import mybir
from concourse.bass_utils import run_bass_kernel_spmd
from concourse.vector_clock import ScopedClock

B, C, HH, WW = 8, 128, 64, 64
N = HH * WW          # 4096
CH = C // 2          # 64
NB = 512             # n-block (one PSUM bank of fp32)
NBLK = N // NB       # 8
MT = 128             # m-tile
NMT = N // MT        # 32
F32 = mybir.dt.float32
F32R = mybir.dt.float32r
BF16 = mybir.dt.bfloat16


def _patched_drain_and_barrier(self, tick_clock, wait_clock):
    # Walrus in this env rejects >1-2 sync waits on the Tile tail Drain
    # ("Too many sync wait commands"). Emit the waits as separate SP
    # instructions, then a bare drain.
    nc = self.nc
    carrier = nc.sync.nop(hint="tail_wait_carrier", nofuse=True)
    wait_clock.add_sem_waits(
        carrier.ins, ScopedClock({None: tick_clock.global_clock})
    )
    waits = list(carrier.ins.sync_info.on_wait)
    carrier.ins.sync_info.on_wait = waits[:1]
    sem_by_name = {h.name: h for h in wait_clock.sems.allocated().values()}
    for w in waits[1:]:
        nc.sync.wait_ge(sem_by_name[w.ant_name], w.wait_value)
    nc.sync.drain()
    nc.all_engine_barrier()
    assert self.sems is not None
    popped = nc._tile_sem_poison_stack.pop()
    assert popped is self._sem_poison
    nc.clear_and_free_semaphores(list(self.sems.allocated().values()))
    nc.all_engine_barrier()


tile.TileContext._drain_and_barrier = _patched_drain_and_barrier


def _split_waits_json(bir_bytes: bytes) -> bytes:
    """Walrus here supports only one sync-wait command per instruction.
    Hoist extra waits onto same-engine NoOps inserted just before."""
    bir = json.loads(bir_bytes)
    for func in bir["functions"]:
        for blk in func["blocks"]:
            new = []
            for ins in blk["instructions"]:
                si = ins.get("sync_info")
                waits = si.get("on_wait", []) if si else []
                if len(waits) > 1:
                    for k, w in enumerate(waits[:-1]):
                        nop = {
                            "engine": ins["engine"],
                            "ins": [],
                            "outs": [],
                            "name": f'{ins["name"]}.w{k}',
                            "opcode": "NoOp",
                            "sync_info": {"on_update": [], "on_wait": [w]},
                            "text_hint": "wait_split",
                        }
                        if ins.get("debug") is not None:
                            nop["debug"] = ins["debug"]
                        new.append(nop)
                    si["on_wait"] = waits[-1:]
                new.append(ins)
            blk["instructions"] = new
    return json.dumps(bir).encode()


def _patched_to_json_bytes(self) -> bytes:
    return _split_waits_json(mybir.module_to_json_bytes(self.m))


def build_nc(ve_groups: int = 12, gp_groups: int = 10) -> bass.Bass:
    """Of the 16 m-tile pairs per n-block, ve_groups accumulate the softmax
    denominator on VectorE, gp_groups on GpSimd, the rest via PE
    ones-matmuls (bf16 rhs, fp32 PSUM accumulation)."""
    nc = bass.Bass(trn_type="TRN2")
    nc.to_json_bytes = types.MethodType(_patched_to_json_bytes, nc)
    x = nc.dram_tensor("x", [C, N], F32, kind="ExternalInput")
    wfT = nc.dram_tensor("wfT", [C, CH], F32, kind="ExternalInput")  # Wf^T
    wgT = nc.dram_tensor("wgT", [C, CH], F32, kind="ExternalInput")  # Wg^T
    bf = nc.dram_tensor("bf", [CH, 1], F32, kind="ExternalInput")
    bg = nc.dram_tensor("bg", [CH, 1], F32, kind="ExternalInput")
    whT = nc.dram_tensor("whT", [C, C], F32, kind="ExternalInput")   # Wh^T
    bh = nc.dram_tensor("bh", [1, C], F32, kind="ExternalInput")
    gamma = nc.dram_tensor("gamma", [1, 1], F32, kind="ExternalInput")
    out = nc.dram_tensor("out", [C, N], F32, kind="ExternalOutput")
    dscratch = nc.dram_tensor("dscratch", [NBLK, NB], F32)

    NG = NMT // 2  # 16 m-tile pairs ("groups") per n-block

    with tile.TileContext(nc) as tc:
        with (
            tc.tile_pool(name="big", bufs=1) as big,
            tc.tile_pool(name="consts", bufs=1) as consts,
            tc.tile_pool(name="ework", bufs=4) as ework,
            tc.tile_pool(name="dwork", bufs=2) as dwork,
            tc.tile_pool(name="small", bufs=2) as small,
            tc.tile_pool(name="pmm", bufs=2, space="PSUM") as pmm,
            tc.tile_pool(name="po", bufs=1, space="PSUM") as po,
            tc.tile_pool(name="pd", bufs=1, space="PSUM") as pd,
        ):
            # ---- constants / params (matmul operands in fp32r) ----
            wfT_sb = consts.tile([C, CH], F32R)
            nc.gpsimd.dma_start(out=wfT_sb, in_=wfT[:, :])
            wgT_sb = consts.tile([C, CH], F32R)
            nc.gpsimd.dma_start(out=wgT_sb, in_=wgT[:, :])
            whT_sb = consts.tile([C, C], F32R)
            nc.gpsimd.dma_start(out=whT_sb, in_=whT[:, :])
            bf_sb = consts.tile([CH, 1], F32)
            nc.sync.dma_start(out=bf_sb, in_=bf[:, :])
            bg_sb = consts.tile([CH, 1], F32)
            nc.sync.dma_start(out=bg_sb, in_=bg[:, :])
            gamma_sb = consts.tile([1, 1], F32)
            nc.sync.dma_start(out=gamma_sb, in_=gamma[:, :])
            bh_bcast = consts.tile([C, C], F32)
            bh_ap = bh[:, :]
            nc.sync.dma_start(
                out=bh_bcast,
                in_=bass.AP(
                    tensor=bh_ap.tensor,
                    offset=bh_ap.offset,
                    ap=[[0, C]] + list(bh_ap.ap)[1:],
                ),
            )
            ones_col_f = consts.tile([C, 1], F32)
            nc.vector.memset(ones_col_f, 1.0)
            ones_col = consts.tile([C, 1], BF16)
            nc.vector.tensor_copy(ones_col, ones_col_f)

            # ---- x: fp32 copy for the residual, fp32r copy for matmuls ----
            x_sb = big.tile([C, N], F32)
            xr_sb = big.tile([C, N], F32R)
            for i in range(NBLK):
                sl = slice(i * NB, (i + 1) * NB)
                nc.sync.dma_start(out=x_sb[:, sl], in_=x[:, sl])
                nc.gpsimd.dma_start(out=xr_sb[:, sl], in_=x[:, sl])

            # ---- f = Wf x + bf, g = Wg x + bg  -> [64, 4096] each ----
            f_sb = big.tile([CH, N], F32R)
            g_sb = big.tile([CH, N], F32R)
            for i in range(NBLK):
                sl = slice(i * NB, (i + 1) * NB)
                psum_f = pmm.tile([C, NB], F32, tag="mm")
                nc.tensor.matmul(
                    psum_f[0:CH, :], wfT_sb, xr_sb[:, sl], start=True, stop=True
                )
                nc.vector.tensor_scalar_add(f_sb[:, sl], psum_f[0:CH, :], bf_sb)
                psum_g = pmm.tile([C, NB], F32, tag="mm")
                nc.tensor.matmul(
                    psum_g[0:CH, :], wgT_sb, xr_sb[:, sl], start=True, stop=True
                )
                nc.vector.tensor_scalar_add(g_sb[:, sl], psum_g[0:CH, :], bg_sb)

            # ---- hT (h^T stored as 32 column-blocks of [128m, 128c]) ----
            hT_sb = big.tile([C, N], BF16)
            for j in range(NMT):
                sl = slice(j * MT, (j + 1) * MT)
                psum_h = pmm.tile([C, MT], F32, tag="mm")
                nc.tensor.matmul(psum_h, xr_sb[:, sl], whT_sb, start=True, stop=True)
                nc.vector.tensor_add(hT_sb[:, sl], psum_h, bh_bcast)

            # ---- main attention loop: 2 n-blocks per round so each
            # stationary operand (f_j / hT_j) serves 2 back-to-back matmuls
            NPAIR = NBLK // 2
            for pr in range(NPAIR):
                nsl2 = slice(pr * 2 * NB, (pr + 1) * 2 * NB)      # both blocks
                nsl_a = slice(pr * 2 * NB, pr * 2 * NB + NB)
                nsl_b = slice(pr * 2 * NB + NB, (pr + 1) * 2 * NB)
                g2 = g_sb[:, nsl2]
                d_acc = dwork.tile([C, 2 * NB], F32, tag="dacc")
                d_acc2 = dwork.tile([C, 2 * NB], F32, tag="dacc2")
                psum_o = po.tile([C, 2 * NB], F32, tag="o")
                psum_d = pd.tile([1, 2 * NB], F32, tag="d")
                ve_seen = 0
                gp_seen = 0
                pe_seen = 0
                pending = []

                def consume(q, e2q):
                    nonlocal ve_seen, gp_seen, pe_seen
                    mslq = slice(q * MT, (q + 1) * MT)
                    lane = q % 3
                    if lane == 0 and ve_seen < ve_groups:
                        if ve_seen == 0:
                            nc.vector.tensor_copy(d_acc, e2q)
                        else:
                            nc.vector.tensor_add(d_acc, d_acc, e2q)
                        ve_seen += 1
                    elif lane == 1 and gp_seen < gp_groups:
                        if gp_seen == 0:
                            nc.gpsimd.tensor_copy(d_acc2, e2q)
                        else:
                            nc.gpsimd.tensor_add(d_acc2, d_acc2, e2q)
                        gp_seen += 1
                    else:
                        nc.tensor.matmul(
                            psum_d[:, 0:NB],
                            ones_col,
                            e2q[:, 0:NB],
                            start=(pe_seen == 0),
                            stop=False,
                            skip_group_check=True,
                        )
                        nc.tensor.matmul(
                            psum_d[:, NB : 2 * NB],
                            ones_col,
                            e2q[:, NB : 2 * NB],
                            start=(pe_seen == 0),
                            stop=False,
                            skip_group_check=True,
                        )
                        pe_seen += 1
                    nc.tensor.matmul(
                        psum_o[:, 0:NB],
                        hT_sb[:, mslq],
                        e2q[:, 0:NB],
                        start=(q == 0),
                        stop=False,
                        skip_group_check=True,
                    )
                    nc.tensor.matmul(
                        psum_o[:, NB : 2 * NB],
                        hT_sb[:, mslq],
                        e2q[:, NB : 2 * NB],
                        start=(q == 0),
                        stop=(q == NMT - 1),
                        skip_group_check=True,
                    )

                for j in range(NMT):
                    msl = slice(j * MT, (j + 1) * MT)
                    t2 = pmm.tile([C, 2 * NB], F32, tag="mm")
                    nc.tensor.matmul(
                        t2[:, 0:NB], f_sb[:, msl], g2[:, 0:NB], start=True, stop=True
                    )
                    nc.tensor.matmul(
                        t2[:, NB : 2 * NB],
                        f_sb[:, msl],
                        g2[:, NB : 2 * NB],
                        start=True,
                        stop=True,
                    )
                    e2 = ework.tile([C, 2 * NB], BF16, tag="e")
                    nc.scalar.activation(e2, t2, mybir.ActivationFunctionType.Exp)
                    pending.append((j, e2))
                    if len(pending) > 2:
                        consume(*pending.pop(0))
                while pending:
                    consume(*pending.pop(0))
                # merge gpsimd partial, then fold both halves (fp32 LOW_HIGH)
                nc.vector.tensor_add(d_acc, d_acc, d_acc2)
                nc.tensor.matmul(
                    psum_d[:, 0:NB],
                    ones_col_f,
                    d_acc[:, 0:NB],
                    start=(pe_seen == 0),
                    stop=False,
                    skip_group_check=True,
                )
                nc.tensor.matmul(
                    psum_d[:, NB : 2 * NB],
                    ones_col_f,
                    d_acc[:, NB : 2 * NB],
                    start=(pe_seen == 0),
                    stop=True,
                    skip_group_check=True,
                )
                d_f32 = small.tile([1, 2 * NB], F32, tag="drec")
                nc.vector.reciprocal(d_f32, psum_d)
                d_g = small.tile([1, 2 * NB], F32, tag="dg")
                nc.vector.tensor_scalar_mul(d_g, d_f32, gamma_sb)
                # broadcast gamma/d across partitions via a DRAM bounce
                nc.sync.dma_start(out=dscratch[2 * pr : 2 * pr + 2, :], in_=d_g)
                b_sb = small.tile([C, 2 * NB], F32, tag="bsb")
                dsc = dscratch[2 * pr : 2 * pr + 2, :]
                dsc_flat = bass.AP(
                    tensor=dsc.tensor,
                    offset=dsc.offset,
                    ap=[[0, C], [1, 2 * NB]],
                )
                nc.sync.dma_start(out=b_sb, in_=dsc_flat)
                res = small.tile([C, 2 * NB], F32, tag="res")
                nc.vector.tensor_mul(res, psum_o, b_sb)
                nc.vector.tensor_add(res, res, x_sb[:, nsl2])
                nc.sync.dma_start(out=out[:, nsl2], in_=res)

    return nc


_NC = None


def get_nc() -> bass.Bass:
    global _NC
    if _NC is None:
        _NC = build_nc()
    return _NC


def make_in_maps(inputs: dict) -> list[dict]:
    x = np.ascontiguousarray(np.asarray(inputs["x"], dtype=np.float32))
    Wf = np.asarray(inputs["Wf"], dtype=np.float32)
    Wg = np.asarray(inputs["Wg"], dtype=np.float32)
    Wh = np.asarray(inputs["Wh"], dtype=np.float32)
    bf = np.asarray(inputs["bf"], dtype=np.float32)
    bg = np.asarray(inputs["bg"], dtype=np.float32)
    bh = np.asarray(inputs["bh"], dtype=np.float32)
    gamma = np.asarray(inputs["gamma"], dtype=np.float32)

    wfT = np.ascontiguousarray(Wf.T)                                  # [128,64]
    wgT = np.ascontiguousarray(Wg.T)                                  # [128,64]
    whT = np.ascontiguousarray(Wh.T)                                  # [128,128]
    bf_c = np.ascontiguousarray(bf[:, None])                          # [64,1]
    bg_c = np.ascontiguousarray(bg[:, None])                          # [64,1]
    bh_row = np.ascontiguousarray(bh[None, :])                        # [1,128]
    gam = np.ascontiguousarray(gamma.reshape(1, 1))                   # [1,1]

    in_maps = []
    for b in range(B):
        in_maps.append(
            {
                "x": np.ascontiguousarray(x[b].reshape(C, N)),
                "wfT": wfT,
                "wgT": wgT,
                "bf": bf_c,
                "bg": bg_c,
                "whT": whT,
                "bh": bh_row,
                "gamma": gam,
            }
        )
    return in_maps


def kernel(**inputs) -> np.ndarray:
    nc = get_nc()
    in_maps = make_in_maps(inputs)
    res = run_bass_kernel_spmd(nc, in_maps, core_ids=list(range(B)))
    out = np.stack([res.results[b]["out"].reshape(C, HH, WW) for b in range(B)])
    return out.astype(np.float32)


# revision 18
# speedup vs baseline: 1.1173x; 1.1173x over previous
"""SAGAN-style attention (nn_Attention_24927990186686) on 8 TRN2 cores.

reference:
  f = Wf@x+bf  [B,64,N]   g = Wg@x+bg  [B,64,N]   h = Wh@x+bh  [B,128,N]
  s = g^T f    [B,N,N]    beta = softmax(s, -1)
  o[c,n] = sum_m beta[n,m] h[c,m];  out = gamma*o + x     (B=8, N=4096)

Sharding: data-parallel over batch, one batch per core, params replicated.

Per-core algorithm ("orientation B" — score tiles transposed so the softmax
contraction (m) lands on the partition axis, which is what the second matmul
contracts over; no giant transposes needed):
  fg  = [Wf^T|Wg^T]^T @ x + [bf;bg]       [128, 4096]  (f rows 0:64, g 64:128)
  hT_j = (x_tile_j)^T... via matmul(lhsT=x[:,128j:128j+128], rhs=Wh^T) + bh
  per n-block (512 cols):
    for j in 32:  t_j = f_j^T g_blk (PSUM) -> e_j = exp(t_j) (ACT)
                  d_acc += e_j (DVE);  psum_o += hT_j^T e_j (PE, accum)
    d = ones^T d_acc (PE row-reduce) -> recip*gamma (DVE)
    bcast via ones outer-product (PE) -> out = psum_o*bcast + x (DVE)
softmax max-subtraction is skipped: |s| <~ 50 for these input distributions,
exp stays comfortably inside fp32 range, and normalization cancels the shift.
"""

import json
import sys
import types

if "/opt/trn_rl_repo" not in sys.path:
    sys.path.insert(0, "/opt/trn_rl_repo")

import numpy as np

import concourse.bass as bass
import concourse.tile as tile
from concourse import mybir
from concourse.bass_utils import run_bass_kernel_spmd
from concourse.vector_clock import ScopedClock

B, C, HH, WW = 8, 128, 64, 64
N = HH * WW          # 4096
CH = C // 2          # 64
NB = 512             # n-block (one PSUM bank of fp32)
NBLK = N // NB       # 8
MT = 128             # m-tile
NMT = N // MT        # 32
F32 = mybir.dt.float32
F32R = mybir.dt.float32r
BF16 = mybir.dt.bfloat16


def _patched_drain_and_barrier(self, tick_clock, wait_clock):
    # Walrus in this env rejects >1-2 sync waits on the Tile tail Drain
    # ("Too many sync wait commands"). Emit the waits as separate SP
    # instructions, then a bare drain.
    nc = self.nc
    carrier = nc.sync.nop(hint="tail_wait_carrier", nofuse=True)
    wait_clock.add_sem_waits(
        carrier.ins, ScopedClock({None: tick_clock.global_clock})
    )
    waits = list(carrier.ins.sync_info.on_wait)
    carrier.ins.sync_info.on_wait = waits[:1]
    sem_by_name = {h.name: h for h in wait_clock.sems.allocated().values()}
    for w in waits[1:]:
        nc.sync.wait_ge(sem_by_name[w.ant_name], w.wait_value)
    nc.sync.drain()
    nc.all_engine_barrier()
    assert self.sems is not None
    popped = nc._tile_sem_poison_stack.pop()
    assert popped is self._sem_poison
    nc.clear_and_free_semaphores(list(self.sems.allocated().values()))
    nc.all_engine_barrier()


tile.TileContext._drain_and_barrier = _patched_drain_and_barrier


def _split_waits_json(bir_bytes: bytes) -> bytes:
    """Walrus here supports only one sync-wait command per instruction.
    Hoist extra waits onto same-engine NoOps inserted just before."""
    bir = json.loads(bir_bytes)
    for func in bir["functions"]:
        for blk in func["blocks"]:
            new = []
            for ins in blk["instructions"]:
                si = ins.get("sync_info")
                waits = si.get("on_wait", []) if si else []
                if len(waits) > 1:
                    for k, w in enumerate(waits[:-1]):
                        nop = {
                            "engine": ins["engine"],
                            "ins": [],
                            "outs": [],
                            "name": f'{ins["name"]}.w{k}',
                            "opcode": "NoOp",
                            "sync_info": {"on_update": [], "on_wait": [w]},
                            "text_hint": "wait_split",
                        }
                        if ins.get("debug") is not None:
                            nop["debug"] = ins["debug"]
                        new.append(nop)
                    si["on_wait"] = waits[-1:]
                new.append(ins)
            blk["instructions"] = new
    return json.dumps(bir).encode()


def _patched_to_json_bytes(self) -> bytes:
    return _split_waits_json(mybir.module_to_json_bytes(self.m))


def build_nc(ve_groups: int = 12, gp_groups: int = 8) -> bass.Bass:
    """Of the 16 m-tile pairs per n-block, ve_groups accumulate the softmax
    denominator on VectorE, gp_groups on GpSimd, the rest via PE
    ones-matmuls (bf16 rhs, fp32 PSUM accumulation)."""
    nc = bass.Bass(trn_type="TRN2")
    nc.to_json_bytes = types.MethodType(_patched_to_json_bytes, nc)
    x = nc.dram_tensor("x", [C, N], F32, kind="ExternalInput")
    wfT = nc.dram_tensor("wfT", [C, CH], F32, kind="ExternalInput")  # Wf^T
    wgT = nc.dram_tensor("wgT", [C, CH], F32, kind="ExternalInput")  # Wg^T
    bf = nc.dram_tensor("bf", [CH, 1], F32, kind="ExternalInput")
    bg = nc.dram_tensor("bg", [CH, 1], F32, kind="ExternalInput")
    whT = nc.dram_tensor("whT", [C, C], F32, kind="ExternalInput")   # Wh^T
    bh = nc.dram_tensor("bh", [1, C], F32, kind="ExternalInput")
    gamma = nc.dram_tensor("gamma", [1, 1], F32, kind="ExternalInput")
    out = nc.dram_tensor("out", [C, N], F32, kind="ExternalOutput")
    dscratch = nc.dram_tensor("dscratch", [NBLK, NB], F32)

    NG = NMT // 2  # 16 m-tile pairs ("groups") per n-block

    with tile.TileContext(nc) as tc:
        with (
            tc.tile_pool(name="big", bufs=1) as big,
            tc.tile_pool(name="consts", bufs=1) as consts,
            tc.tile_pool(name="ework", bufs=6) as ework,
            tc.tile_pool(name="dwork", bufs=2) as dwork,
            tc.tile_pool(name="small", bufs=2) as small,
            tc.tile_pool(name="pmm", bufs=2, space="PSUM") as pmm,
            tc.tile_pool(name="po", bufs=1, space="PSUM") as po,
            tc.tile_pool(name="pd", bufs=1, space="PSUM") as pd,
        ):
            # ---- constants / params (matmul operands in fp32r) ----
            wfT_sb = consts.tile([C, CH], F32R)
            nc.gpsimd.dma_start(out=wfT_sb, in_=wfT[:, :])
            wgT_sb = consts.tile([C, CH], F32R)
            nc.gpsimd.dma_start(out=wgT_sb, in_=wgT[:, :])
            whT_sb = consts.tile([C, C], F32R)
            nc.gpsimd.dma_start(out=whT_sb, in_=whT[:, :])
            bf_sb = consts.tile([CH, 1], F32)
            nc.sync.dma_start(out=bf_sb, in_=bf[:, :])
            bg_sb = consts.tile([CH, 1], F32)
            nc.sync.dma_start(out=bg_sb, in_=bg[:, :])
            gamma_sb = consts.tile([1, 1], F32)
            nc.sync.dma_start(out=gamma_sb, in_=gamma[:, :])
            bh_bcast = consts.tile([C, C], F32)
            bh_ap = bh[:, :]
            nc.sync.dma_start(
                out=bh_bcast,
                in_=bass.AP(
                    tensor=bh_ap.tensor,
                    offset=bh_ap.offset,
                    ap=[[0, C]] + list(bh_ap.ap)[1:],
                ),
            )
            ones_col_f = consts.tile([C, 1], F32)
            nc.vector.memset(ones_col_f, 1.0)
            ones_col = consts.tile([C, 1], BF16)
            nc.vector.tensor_copy(ones_col, ones_col_f)

            # ---- x: fp32 copy for the residual, fp32r copy for matmuls ----
            x_sb = big.tile([C, N], F32)
            xr_sb = big.tile([C, N], F32R)
            for i in range(NBLK):
                sl = slice(i * NB, (i + 1) * NB)
                nc.sync.dma_start(out=x_sb[:, sl], in_=x[:, sl])
                nc.gpsimd.dma_start(out=xr_sb[:, sl], in_=x[:, sl])

            # ---- f = Wf x + bf, g = Wg x + bg  -> [64, 4096] each ----
            f_sb = big.tile([CH, N], F32R)
            g_sb = big.tile([CH, N], F32R)
            for i in range(NBLK):
                sl = slice(i * NB, (i + 1) * NB)
                psum_f = pmm.tile([C, NB], F32, tag="mm")
                nc.tensor.matmul(
                    psum_f[0:CH, :], wfT_sb, xr_sb[:, sl], start=True, stop=True
                )
                nc.vector.tensor_scalar_add(f_sb[:, sl], psum_f[0:CH, :], bf_sb)
                psum_g = pmm.tile([C, NB], F32, tag="mm")
                nc.tensor.matmul(
                    psum_g[0:CH, :], wgT_sb, xr_sb[:, sl], start=True, stop=True
                )
                nc.vector.tensor_scalar_add(g_sb[:, sl], psum_g[0:CH, :], bg_sb)

            # ---- hT (h^T stored as 32 column-blocks of [128m, 128c]) ----
            hT_sb = big.tile([C, N], BF16)
            for j in range(NMT):
                sl = slice(j * MT, (j + 1) * MT)
                psum_h = pmm.tile([C, MT], F32, tag="mm")
                nc.tensor.matmul(psum_h, xr_sb[:, sl], whT_sb, start=True, stop=True)
                nc.vector.tensor_add(hT_sb[:, sl], psum_h, bh_bcast)

            # ---- main attention loop: 2 n-blocks per round so each
            # stationary operand (f_j / hT_j) serves 2 back-to-back matmuls
            NPAIR = NBLK // 2
            for pr in range(NPAIR):
                nsl2 = slice(pr * 2 * NB, (pr + 1) * 2 * NB)      # both blocks
                nsl_a = slice(pr * 2 * NB, pr * 2 * NB + NB)
                nsl_b = slice(pr * 2 * NB + NB, (pr + 1) * 2 * NB)
                g2 = g_sb[:, nsl2]
                d_acc = dwork.tile([C, 2 * NB], F32, tag="dacc")
                d_acc2 = dwork.tile([C, 2 * NB], F32, tag="dacc2")
                psum_o = po.tile([C, 2 * NB], F32, tag="o")
                psum_d = pd.tile([1, 2 * NB], F32, tag="d")
                ve_seen = 0
                gp_seen = 0
                pe_seen = 0
                pending = []

                def consume(q, e2q):
                    nonlocal ve_seen, gp_seen, pe_seen
                    mslq = slice(q * MT, (q + 1) * MT)
                    lane = q % 3
                    if lane == 0 and ve_seen < ve_groups:
                        if ve_seen == 0:
                            nc.vector.tensor_copy(d_acc, e2q)
                        else:
                            nc.vector.tensor_add(d_acc, d_acc, e2q)
                        ve_seen += 1
                    elif lane == 1 and gp_seen < gp_groups:
                        if gp_seen == 0:
                            nc.gpsimd.tensor_copy(d_acc2, e2q)
                        else:
                            nc.gpsimd.tensor_add(d_acc2, d_acc2, e2q)
                        gp_seen += 1
                    else:
                        nc.tensor.matmul(
                            psum_d[:, 0:NB],
                            ones_col,
                            e2q[:, 0:NB],
                            start=(pe_seen == 0),
                            stop=False,
                            skip_group_check=True,
                        )
                        nc.tensor.matmul(
                            psum_d[:, NB : 2 * NB],
                            ones_col,
                            e2q[:, NB : 2 * NB],
                            start=(pe_seen == 0),
                            stop=False,
                            skip_group_check=True,
                        )
                        pe_seen += 1
                    nc.tensor.matmul(
                        psum_o[:, 0:NB],
                        hT_sb[:, mslq],
                        e2q[:, 0:NB],
                        start=(q == 0),
                        stop=False,
                        skip_group_check=True,
                    )
                    nc.tensor.matmul(
                        psum_o[:, NB : 2 * NB],
                        hT_sb[:, mslq],
                        e2q[:, NB : 2 * NB],
                        start=(q == 0),
                        stop=(q == NMT - 1),
                        skip_group_check=True,
                    )

                for j0 in range(0, NMT, 2):
                    # two score groups back-to-back: one fp32r stream, then
                    # one bf16 consume stream (fewer PE mode switches)
                    for j in (j0, j0 + 1):
                        msl = slice(j * MT, (j + 1) * MT)
                        t2 = pmm.tile([C, 2 * NB], F32, tag="mm")
                        nc.tensor.matmul(
                            t2[:, 0:NB],
                            f_sb[:, msl],
                            g2[:, 0:NB],
                            start=True,
                            stop=True,
                        )
                        nc.tensor.matmul(
                            t2[:, NB : 2 * NB],
                            f_sb[:, msl],
                            g2[:, NB : 2 * NB],
                            start=True,
                            stop=True,
                        )
                        e2 = ework.tile([C, 2 * NB], BF16, tag="e")
                        nc.scalar.activation(e2, t2, mybir.ActivationFunctionType.Exp)
                        pending.append((j, e2))
                    while len(pending) > 2:
                        consume(*pending.pop(0))
                while pending:
                    consume(*pending.pop(0))
                # merge gpsimd partial, then fold both halves (fp32 LOW_HIGH)
                nc.vector.tensor_add(d_acc, d_acc, d_acc2)
                nc.tensor.matmul(
                    psum_d[:, 0:NB],
                    ones_col_f,
                    d_acc[:, 0:NB],
                    start=(pe_seen == 0),
                    stop=False,
                    skip_group_check=True,
                )
                nc.tensor.matmul(
                    psum_d[:, NB : 2 * NB],
                    ones_col_f,
                    d_acc[:, NB : 2 * NB],
                    start=(pe_seen == 0),
                    stop=True,
                    skip_group_check=True,
                )
                d_f32 = small.tile([1, 2 * NB], F32, tag="drec")
                nc.vector.reciprocal(d_f32, psum_d)
                d_g = small.tile([1, 2 * NB], F32, tag="dg")
                nc.vector.tensor_scalar_mul(d_g, d_f32, gamma_sb)
                # broadcast gamma/d across partitions via a DRAM bounce
                nc.sync.dma_start(out=dscratch[2 * pr : 2 * pr + 2, :], in_=d_g)
                b_sb = small.tile([C, 2 * NB], F32, tag="bsb")
                dsc = dscratch[2 * pr : 2 * pr + 2, :]
                dsc_flat = bass.AP(
                    tensor=dsc.tensor,
                    offset=dsc.offset,
                    ap=[[0, C], [1, 2 * NB]],
                )
                nc.sync.dma_start(out=b_sb, in_=dsc_flat)
                res = small.tile([C, 2 * NB], F32, tag="res")
                nc.vector.tensor_mul(res, psum_o, b_sb)
                nc.vector.tensor_add(res, res, x_sb[:, nsl2])
                nc.sync.dma_start(out=out[:, nsl2], in_=res)

    return nc


_NC = None


def get_nc() -> bass.Bass:
    global _NC
    if _NC is None:
        _NC = build_nc()
    return _NC


def make_in_maps(inputs: dict) -> list[dict]:
    x = np.ascontiguousarray(np.asarray(inputs["x"], dtype=np.float32))
    Wf = np.asarray(inputs["Wf"], dtype=np.float32)
    Wg = np.asarray(inputs["Wg"], dtype=np.float32)
    Wh = np.asarray(inputs["Wh"], dtype=np.float32)
    bf = np.asarray(inputs["bf"], dtype=np.float32)
    bg = np.asarray(inputs["bg"], dtype=np.float32)
    bh = np.asarray(inputs["bh"], dtype=np.float32)
    gamma = np.asarray(inputs["gamma"], dtype=np.float32)

    wfT = np.ascontiguousarray(Wf.T)                                  # [128,64]
    wgT = np.ascontiguousarray(Wg.T)                                  # [128,64]
    whT = np.ascontiguousarray(Wh.T)                                  # [128,128]
    bf_c = np.ascontiguousarray(bf[:, None])                          # [64,1]
    bg_c = np.ascontiguousarray(bg[:, None])                          # [64,1]
    bh_row = np.ascontiguousarray(bh[None, :])                        # [1,128]
    gam = np.ascontiguousarray(gamma.reshape(1, 1))                   # [1,1]

    in_maps = []
    for b in range(B):
        in_maps.append(
            {
                "x": np.ascontiguousarray(x[b].reshape(C, N)),
                "wfT": wfT,
                "wgT": wgT,
                "bf": bf_c,
                "bg": bg_c,
                "whT": whT,
                "bh": bh_row,
                "gamma": gam,
            }
        )
    return in_maps


def kernel(**inputs) -> np.ndarray:
    nc = get_nc()
    in_maps = make_in_maps(inputs)
    res = run_bass_kernel_spmd(nc, in_maps, core_ids=list(range(B)))
    out = np.stack([res.results[b]["out"].reshape(C, HH, WW) for b in range(B)])
    return out.astype(np.float32)


# revision 19
# speedup vs baseline: 1.3535x; 1.2114x over previous
"""SAGAN-style attention (nn_Attention_24927990186686) on 8 TRN2 cores.

reference:
  f = Wf@x+bf  [B,64,N]   g = Wg@x+bg  [B,64,N]   h = Wh@x+bh  [B,128,N]
  s = g^T f    [B,N,N]    beta = softmax(s, -1)
  o[c,n] = sum_m beta[n,m] h[c,m];  out = gamma*o + x     (B=8, N=4096)

Sharding: data-parallel over batch, one batch per core, params replicated.

Per-core algorithm ("orientation B" — score tiles transposed so the softmax
contraction (m) lands on the partition axis, which is what the second matmul
contracts over; no giant transposes needed):
  fg  = [Wf^T|Wg^T]^T @ x + [bf;bg]       [128, 4096]  (f rows 0:64, g 64:128)
  hT_j = (x_tile_j)^T... via matmul(lhsT=x[:,128j:128j+128], rhs=Wh^T) + bh
  per n-block (512 cols):
    for j in 32:  t_j = f_j^T g_blk (PSUM) -> e_j = exp(t_j) (ACT)
                  d_acc += e_j (DVE);  psum_o += hT_j^T e_j (PE, accum)
    d = ones^T d_acc (PE row-reduce) -> recip*gamma (DVE)
    bcast via ones outer-product (PE) -> out = psum_o*bcast + x (DVE)
softmax max-subtraction is skipped: |s| <~ 50 for these input distributions,
exp stays comfortably inside fp32 range, and normalization cancels the shift.
"""

import json
import sys
import types

if "/opt/trn_rl_repo" not in sys.path:
    sys.path.insert(0, "/opt/trn_rl_repo")

import numpy as np

import concourse.bass as bass
import concourse.tile as tile
from concourse import mybir
from concourse.bass_utils import run_bass_kernel_spmd
from concourse.vector_clock import ScopedClock

B, C, HH, WW = 8, 128, 64, 64
N = HH * WW          # 4096
CH = C // 2          # 64
NB = 512             # n-block (one PSUM bank of fp32)
NBLK = N // NB       # 8
MT = 128             # m-tile
NMT = N // MT        # 32
F32 = mybir.dt.float32
F32R = mybir.dt.float32r
BF16 = mybir.dt.bfloat16


def _patched_drain_and_barrier(self, tick_clock, wait_clock):
    # Walrus in this env rejects >1-2 sync waits on the Tile tail Drain
    # ("Too many sync wait commands"). Emit the waits as separate SP
    # instructions, then a bare drain.
    nc = self.nc
    carrier = nc.sync.nop(hint="tail_wait_carrier", nofuse=True)
    wait_clock.add_sem_waits(
        carrier.ins, ScopedClock({None: tick_clock.global_clock})
    )
    waits = list(carrier.ins.sync_info.on_wait)
    carrier.ins.sync_info.on_wait = waits[:1]
    sem_by_name = {h.name: h for h in wait_clock.sems.allocated().values()}
    for w in waits[1:]:
        nc.sync.wait_ge(sem_by_name[w.ant_name], w.wait_value)
    nc.sync.drain()
    nc.all_engine_barrier()
    assert self.sems is not None
    popped = nc._tile_sem_poison_stack.pop()
    assert popped is self._sem_poison
    nc.clear_and_free_semaphores(list(self.sems.allocated().values()))
    nc.all_engine_barrier()


tile.TileContext._drain_and_barrier = _patched_drain_and_barrier


def _split_waits_json(bir_bytes: bytes) -> bytes:
    """Walrus here supports only one sync-wait command per instruction.
    Hoist extra waits onto same-engine NoOps inserted just before."""
    bir = json.loads(bir_bytes)
    for func in bir["functions"]:
        for blk in func["blocks"]:
            new = []
            for ins in blk["instructions"]:
                si = ins.get("sync_info")
                waits = si.get("on_wait", []) if si else []
                if len(waits) > 1:
                    for k, w in enumerate(waits[:-1]):
                        nop = {
                            "engine": ins["engine"],
                            "ins": [],
                            "outs": [],
                            "name": f'{ins["name"]}.w{k}',
                            "opcode": "NoOp",
                            "sync_info": {"on_update": [], "on_wait": [w]},
                            "text_hint": "wait_split",
                        }
                        if ins.get("debug") is not None:
                            nop["debug"] = ins["debug"]
                        new.append(nop)
                    si["on_wait"] = waits[-1:]
                new.append(ins)
            blk["instructions"] = new
    return json.dumps(bir).encode()


def _patched_to_json_bytes(self) -> bytes:
    return _split_waits_json(mybir.module_to_json_bytes(self.m))


def build_nc(ve_groups: int = 12, gp_groups: int = 8) -> bass.Bass:
    """Of the 16 m-tile pairs per n-block, ve_groups accumulate the softmax
    denominator on VectorE, gp_groups on GpSimd, the rest via PE
    ones-matmuls (bf16 rhs, fp32 PSUM accumulation)."""
    nc = bass.Bass(trn_type="TRN2")
    nc.to_json_bytes = types.MethodType(_patched_to_json_bytes, nc)
    x = nc.dram_tensor("x", [C, N], F32, kind="ExternalInput")
    wfT = nc.dram_tensor("wfT", [C, CH], F32, kind="ExternalInput")  # Wf^T
    wgT = nc.dram_tensor("wgT", [C, CH], F32, kind="ExternalInput")  # Wg^T
    bf = nc.dram_tensor("bf", [CH, 1], F32, kind="ExternalInput")
    bg = nc.dram_tensor("bg", [CH, 1], F32, kind="ExternalInput")
    whT = nc.dram_tensor("whT", [C, C], F32, kind="ExternalInput")   # Wh^T
    bh = nc.dram_tensor("bh", [1, C], F32, kind="ExternalInput")
    gamma = nc.dram_tensor("gamma", [1, 1], F32, kind="ExternalInput")
    out = nc.dram_tensor("out", [C, N], F32, kind="ExternalOutput")
    dscratch = nc.dram_tensor("dscratch", [NBLK, NB], F32)

    NG = NMT // 2  # 16 m-tile pairs ("groups") per n-block

    with tile.TileContext(nc) as tc:
        with (
            tc.tile_pool(name="big", bufs=1) as big,
            tc.tile_pool(name="consts", bufs=1) as consts,
            tc.tile_pool(name="ework", bufs=6) as ework,
            tc.tile_pool(name="dwork", bufs=2) as dwork,
            tc.tile_pool(name="small", bufs=2) as small,
            tc.tile_pool(name="pmm", bufs=2, space="PSUM") as pmm,
            tc.tile_pool(name="po", bufs=1, space="PSUM") as po,
            tc.tile_pool(name="pd", bufs=1, space="PSUM") as pd,
        ):
            # ---- constants / params (matmul operands in fp32r) ----
            wfT_sb = consts.tile([C, CH], F32R)
            nc.gpsimd.dma_start(out=wfT_sb, in_=wfT[:, :])
            wgT_sb = consts.tile([C, CH], F32R)
            nc.gpsimd.dma_start(out=wgT_sb, in_=wgT[:, :])
            whT_sb = consts.tile([C, C], F32R)
            nc.gpsimd.dma_start(out=whT_sb, in_=whT[:, :])
            bf_sb = consts.tile([CH, 1], F32)
            nc.sync.dma_start(out=bf_sb, in_=bf[:, :])
            bg_sb = consts.tile([CH, 1], F32)
            nc.sync.dma_start(out=bg_sb, in_=bg[:, :])
            gamma_sb = consts.tile([1, 1], F32)
            nc.sync.dma_start(out=gamma_sb, in_=gamma[:, :])
            bh_bcast = consts.tile([C, C], F32)
            bh_ap = bh[:, :]
            nc.sync.dma_start(
                out=bh_bcast,
                in_=bass.AP(
                    tensor=bh_ap.tensor,
                    offset=bh_ap.offset,
                    ap=[[0, C]] + list(bh_ap.ap)[1:],
                ),
            )
            ones_col_f = consts.tile([C, 1], F32)
            nc.vector.memset(ones_col_f, 1.0)
            ones_col = consts.tile([C, 1], BF16)
            nc.vector.tensor_copy(ones_col, ones_col_f)

            # ---- x: fp32 copy for the residual, fp32r copy for matmuls ----
            x_sb = big.tile([C, N], F32)
            xr_sb = big.tile([C, N], F32R)
            for i in range(NBLK):
                sl = slice(i * NB, (i + 1) * NB)
                nc.sync.dma_start(out=x_sb[:, sl], in_=x[:, sl])
                nc.gpsimd.dma_start(out=xr_sb[:, sl], in_=x[:, sl])

            # ---- f = Wf x + bf, g = Wg x + bg  -> [64, 4096] each ----
            f_dup = big.tile([C, N], F32R)
            g_dup = big.tile([C, N], F32R)
            for i in range(NBLK):
                sl = slice(i * NB, (i + 1) * NB)
                psum_f = pmm.tile([C, NB], F32, tag="mm")
                nc.tensor.matmul(
                    psum_f[0:CH, :], wfT_sb, xr_sb[:, sl], start=True, stop=True
                )
                nc.vector.tensor_scalar_add(f_dup[0:CH, sl], psum_f[0:CH, :], bf_sb)
                psum_g = pmm.tile([C, NB], F32, tag="mm")
                nc.tensor.matmul(
                    psum_g[0:CH, :], wgT_sb, xr_sb[:, sl], start=True, stop=True
                )
                nc.vector.tensor_scalar_add(g_dup[0:CH, sl], psum_g[0:CH, :], bg_sb)

            # replicate f/g into partitions 64-127 so score matmuls can run
            # as two concurrent row-groups of the PE array
            nc.sync.dma_start(out=f_dup[CH:C, :], in_=f_dup[0:CH, :])
            nc.sync.dma_start(out=g_dup[CH:C, :], in_=g_dup[0:CH, :])

            # ---- hT (h^T stored as 32 column-blocks of [128m, 128c]) ----
            hT_sb = big.tile([C, N], BF16)
            for j in range(NMT):
                sl = slice(j * MT, (j + 1) * MT)
                psum_h = pmm.tile([C, MT], F32, tag="mm")
                nc.tensor.matmul(psum_h, xr_sb[:, sl], whT_sb, start=True, stop=True)
                nc.vector.tensor_add(hT_sb[:, sl], psum_h, bh_bcast)

            # ---- main attention loop: 2 n-blocks per round so each
            # stationary operand (f_j / hT_j) serves 2 back-to-back matmuls
            NPAIR = NBLK // 2
            for pr in range(NPAIR):
                nsl2 = slice(pr * 2 * NB, (pr + 1) * 2 * NB)      # both blocks
                nsl_a = slice(pr * 2 * NB, pr * 2 * NB + NB)
                nsl_b = slice(pr * 2 * NB + NB, (pr + 1) * 2 * NB)
                d_acc = dwork.tile([C, 2 * NB], F32, tag="dacc")
                d_acc2 = dwork.tile([C, 2 * NB], F32, tag="dacc2")
                psum_o = po.tile([C, 2 * NB], F32, tag="o")
                psum_d = pd.tile([1, 2 * NB], F32, tag="d")
                ve_seen = 0
                gp_seen = 0
                pe_seen = 0
                pending = []

                def consume(q, e2q):
                    nonlocal ve_seen, gp_seen, pe_seen
                    mslq = slice(q * MT, (q + 1) * MT)
                    lane = q % 3
                    if lane == 0 and ve_seen < ve_groups:
                        if ve_seen == 0:
                            nc.vector.tensor_copy(d_acc, e2q)
                        else:
                            nc.vector.tensor_add(d_acc, d_acc, e2q)
                        ve_seen += 1
                    elif lane == 1 and gp_seen < gp_groups:
                        if gp_seen == 0:
                            nc.gpsimd.tensor_copy(d_acc2, e2q)
                        else:
                            nc.gpsimd.tensor_add(d_acc2, d_acc2, e2q)
                        gp_seen += 1
                    else:
                        nc.tensor.matmul(
                            psum_d[:, 0:NB],
                            ones_col,
                            e2q[:, 0:NB],
                            start=(pe_seen == 0),
                            stop=False,
                            skip_group_check=True,
                        )
                        nc.tensor.matmul(
                            psum_d[:, NB : 2 * NB],
                            ones_col,
                            e2q[:, NB : 2 * NB],
                            start=(pe_seen == 0),
                            stop=False,
                            skip_group_check=True,
                        )
                        pe_seen += 1
                    nc.tensor.matmul(
                        psum_o[:, 0:NB],
                        hT_sb[:, mslq],
                        e2q[:, 0:NB],
                        start=(q == 0),
                        stop=False,
                        skip_group_check=True,
                    )
                    nc.tensor.matmul(
                        psum_o[:, NB : 2 * NB],
                        hT_sb[:, mslq],
                        e2q[:, NB : 2 * NB],
                        start=(q == 0),
                        stop=(q == NMT - 1),
                        skip_group_check=True,
                    )

                for j0 in range(0, NMT, 2):
                    # two score groups back-to-back: one fp32r stream, then
                    # one bf16 consume stream (fewer PE mode switches)
                    for j in (j0, j0 + 1):
                        msl = slice(j * MT, (j + 1) * MT)
                        t2 = pmm.tile([C, 2 * NB], F32, tag="mm")
                        nc.tensor.matmul(
                            t2[:, 0:NB],
                            f_dup[0:CH, msl],
                            g_dup[0:CH, nsl_a],
                            start=True,
                            stop=True,
                        )
                        nc.tensor.matmul(
                            t2[:, NB : 2 * NB],
                            f_dup[CH:C, msl],
                            g_dup[CH:C, nsl_b],
                            start=True,
                            stop=True,
                        )
                        e2 = ework.tile([C, 2 * NB], BF16, tag="e")
                        nc.scalar.activation(e2, t2, mybir.ActivationFunctionType.Exp)
                        pending.append((j, e2))
                    while len(pending) > 2:
                        consume(*pending.pop(0))
                while pending:
                    consume(*pending.pop(0))
                # merge gpsimd partial, then fold both halves (fp32 LOW_HIGH)
                nc.vector.tensor_add(d_acc, d_acc, d_acc2)
                nc.tensor.matmul(
                    psum_d[:, 0:NB],
                    ones_col_f,
                    d_acc[:, 0:NB],
                    start=(pe_seen == 0),
                    stop=False,
                    skip_group_check=True,
                )
                nc.tensor.matmul(
                    psum_d[:, NB : 2 * NB],
                    ones_col_f,
                    d_acc[:, NB : 2 * NB],
                    start=(pe_seen == 0),
                    stop=True,
                    skip_group_check=True,
                )
                d_f32 = small.tile([1, 2 * NB], F32, tag="drec")
                nc.vector.reciprocal(d_f32, psum_d)
                d_g = small.tile([1, 2 * NB], F32, tag="dg")
                nc.vector.tensor_scalar_mul(d_g, d_f32, gamma_sb)
                # broadcast gamma/d across partitions via a DRAM bounce
                nc.sync.dma_start(out=dscratch[2 * pr : 2 * pr + 2, :], in_=d_g)
                b_sb = small.tile([C, 2 * NB], F32, tag="bsb")
                dsc = dscratch[2 * pr : 2 * pr + 2, :]
                dsc_flat = bass.AP(
                    tensor=dsc.tensor,
                    offset=dsc.offset,
                    ap=[[0, C], [1, 2 * NB]],
                )
                nc.sync.dma_start(out=b_sb, in_=dsc_flat)
                res = small.tile([C, 2 * NB], F32, tag="res")
                nc.vector.tensor_mul(res, psum_o, b_sb)
                nc.vector.tensor_add(res, res, x_sb[:, nsl2])
                nc.sync.dma_start(out=out[:, nsl2], in_=res)

    return nc


_NC = None


def get_nc() -> bass.Bass:
    global _NC
    if _NC is None:
        _NC = build_nc()
    return _NC


def make_in_maps(inputs: dict) -> list[dict]:
    x = np.ascontiguousarray(np.asarray(inputs["x"], dtype=np.float32))
    Wf = np.asarray(inputs["Wf"], dtype=np.float32)
    Wg = np.asarray(inputs["Wg"], dtype=np.float32)
    Wh = np.asarray(inputs["Wh"], dtype=np.float32)
    bf = np.asarray(inputs["bf"], dtype=np.float32)
    bg = np.asarray(inputs["bg"], dtype=np.float32)
    bh = np.asarray(inputs["bh"], dtype=np.float32)
    gamma = np.asarray(inputs["gamma"], dtype=np.float32)

    wfT = np.ascontiguousarray(Wf.T)                                  # [128,64]
    wgT = np.ascontiguousarray(Wg.T)                                  # [128,64]
    whT = np.ascontiguousarray(Wh.T)                                  # [128,128]
    bf_c = np.ascontiguousarray(bf[:, None])                          # [64,1]
    bg_c = np.ascontiguousarray(bg[:, None])                          # [64,1]
    bh_row = np.ascontiguousarray(bh[None, :])                        # [1,128]
    gam = np.ascontiguousarray(gamma.reshape(1, 1))                   # [1,1]

    in_maps = []
    for b in range(B):
        in_maps.append(
            {
                "x": np.ascontiguousarray(x[b].reshape(C, N)),
                "wfT": wfT,
                "wgT": wgT,
                "bf": bf_c,
                "bg": bg_c,
                "whT": whT,
                "bh": bh_row,
                "gamma": gam,
            }
        )
    return in_maps


def kernel(**inputs) -> np.ndarray:
    nc = get_nc()
    in_maps = make_in_maps(inputs)
    res = run_bass_kernel_spmd(nc, in_maps, core_ids=list(range(B)))
    out = np.stack([res.results[b]["out"].reshape(C, HH, WW) for b in range(B)])
    return out.astype(np.float32)


# revision 20
# speedup vs baseline: 1.4638x; 1.0815x over previous
"""SAGAN-style attention (nn_Attention_24927990186686) on 8 TRN2 cores.

reference:
  f = Wf@x+bf  [B,64,N]   g = Wg@x+bg  [B,64,N]   h = Wh@x+bh  [B,128,N]
  s = g^T f    [B,N,N]    beta = softmax(s, -1)
  o[c,n] = sum_m beta[n,m] h[c,m];  out = gamma*o + x     (B=8, N=4096)

Sharding: data-parallel over batch, one batch per core, params replicated.

Per-core algorithm ("orientation B" — score tiles transposed so the softmax
contraction (m) lands on the partition axis, which is what the second matmul
contracts over; no giant transposes needed):
  fg  = [Wf^T|Wg^T]^T @ x + [bf;bg]       [128, 4096]  (f rows 0:64, g 64:128)
  hT_j = (x_tile_j)^T... via matmul(lhsT=x[:,128j:128j+128], rhs=Wh^T) + bh
  per n-block (512 cols):
    for j in 32:  t_j = f_j^T g_blk (PSUM) -> e_j = exp(t_j) (ACT)
                  d_acc += e_j (DVE);  psum_o += hT_j^T e_j (PE, accum)
    d = ones^T d_acc (PE row-reduce) -> recip*gamma (DVE)
    bcast via ones outer-product (PE) -> out = psum_o*bcast + x (DVE)
softmax max-subtraction is skipped: |s| <~ 50 for these input distributions,
exp stays comfortably inside fp32 range, and normalization cancels the shift.
"""

import json
import sys
import types

if "/opt/trn_rl_repo" not in sys.path:
    sys.path.insert(0, "/opt/trn_rl_repo")

import numpy as np

import concourse.bass as bass
import concourse.tile as tile
from concourse import mybir
from concourse.bass_utils import run_bass_kernel_spmd
from concourse.vector_clock import ScopedClock

B, C, HH, WW = 8, 128, 64, 64
N = HH * WW          # 4096
CH = C // 2          # 64
NB = 512             # n-block (one PSUM bank of fp32)
NBLK = N // NB       # 8
MT = 128             # m-tile
NMT = N // MT        # 32
F32 = mybir.dt.float32
F32R = mybir.dt.float32r
BF16 = mybir.dt.bfloat16


def _patched_drain_and_barrier(self, tick_clock, wait_clock):
    # Walrus in this env rejects >1-2 sync waits on the Tile tail Drain
    # ("Too many sync wait commands"). Emit the waits as separate SP
    # instructions, then a bare drain.
    nc = self.nc
    carrier = nc.sync.nop(hint="tail_wait_carrier", nofuse=True)
    wait_clock.add_sem_waits(
        carrier.ins, ScopedClock({None: tick_clock.global_clock})
    )
    waits = list(carrier.ins.sync_info.on_wait)
    carrier.ins.sync_info.on_wait = waits[:1]
    sem_by_name = {h.name: h for h in wait_clock.sems.allocated().values()}
    for w in waits[1:]:
        nc.sync.wait_ge(sem_by_name[w.ant_name], w.wait_value)
    nc.sync.drain()
    nc.all_engine_barrier()
    assert self.sems is not None
    popped = nc._tile_sem_poison_stack.pop()
    assert popped is self._sem_poison
    nc.clear_and_free_semaphores(list(self.sems.allocated().values()))
    nc.all_engine_barrier()


tile.TileContext._drain_and_barrier = _patched_drain_and_barrier


def _split_waits_json(bir_bytes: bytes) -> bytes:
    """Walrus here supports only one sync-wait command per instruction.
    Hoist extra waits onto same-engine NoOps inserted just before."""
    bir = json.loads(bir_bytes)
    for func in bir["functions"]:
        for blk in func["blocks"]:
            new = []
            for ins in blk["instructions"]:
                si = ins.get("sync_info")
                waits = si.get("on_wait", []) if si else []
                if len(waits) > 1:
                    for k, w in enumerate(waits[:-1]):
                        nop = {
                            "engine": ins["engine"],
                            "ins": [],
                            "outs": [],
                            "name": f'{ins["name"]}.w{k}',
                            "opcode": "NoOp",
                            "sync_info": {"on_update": [], "on_wait": [w]},
                            "text_hint": "wait_split",
                        }
                        if ins.get("debug") is not None:
                            nop["debug"] = ins["debug"]
                        new.append(nop)
                    si["on_wait"] = waits[-1:]
                new.append(ins)
            blk["instructions"] = new
    return json.dumps(bir).encode()


def _patched_to_json_bytes(self) -> bytes:
    return _split_waits_json(mybir.module_to_json_bytes(self.m))


def build_nc(ve_groups: int = 12, gp_groups: int = 8) -> bass.Bass:
    """Of the 16 m-tile pairs per n-block, ve_groups accumulate the softmax
    denominator on VectorE, gp_groups on GpSimd, the rest via PE
    ones-matmuls (bf16 rhs, fp32 PSUM accumulation)."""
    nc = bass.Bass(trn_type="TRN2")
    nc.to_json_bytes = types.MethodType(_patched_to_json_bytes, nc)
    x = nc.dram_tensor("x", [C, N], F32, kind="ExternalInput")
    wfT = nc.dram_tensor("wfT", [C, CH], F32, kind="ExternalInput")  # Wf^T
    wgT = nc.dram_tensor("wgT", [C, CH], F32, kind="ExternalInput")  # Wg^T
    bf = nc.dram_tensor("bf", [CH, 1], F32, kind="ExternalInput")
    bg = nc.dram_tensor("bg", [CH, 1], F32, kind="ExternalInput")
    whT = nc.dram_tensor("whT", [C, C], F32, kind="ExternalInput")   # Wh^T
    bh = nc.dram_tensor("bh", [1, C], F32, kind="ExternalInput")
    gamma = nc.dram_tensor("gamma", [1, 1], F32, kind="ExternalInput")
    out = nc.dram_tensor("out", [C, N], F32, kind="ExternalOutput")
    dscratch = nc.dram_tensor("dscratch", [NBLK, NB], F32)

    NG = NMT // 2  # 16 m-tile pairs ("groups") per n-block

    with tile.TileContext(nc) as tc:
        with (
            tc.tile_pool(name="big", bufs=1) as big,
            tc.tile_pool(name="consts", bufs=1) as consts,
            tc.tile_pool(name="ework", bufs=6) as ework,
            tc.tile_pool(name="dwork", bufs=2) as dwork,
            tc.tile_pool(name="small", bufs=2) as small,
            tc.tile_pool(name="pmm", bufs=2, space="PSUM") as pmm,
            tc.tile_pool(name="po", bufs=1, space="PSUM") as po,
            tc.tile_pool(name="pd", bufs=1, space="PSUM") as pd,
        ):
            # ---- constants / params (matmul operands in fp32r) ----
            wfT_sb = consts.tile([C, CH], F32R)
            nc.gpsimd.dma_start(out=wfT_sb, in_=wfT[:, :])
            wgT_sb = consts.tile([C, CH], F32R)
            nc.gpsimd.dma_start(out=wgT_sb, in_=wgT[:, :])
            whT_sb = consts.tile([C, C], F32R)
            nc.gpsimd.dma_start(out=whT_sb, in_=whT[:, :])
            bf_sb = consts.tile([CH, 1], F32)
            nc.sync.dma_start(out=bf_sb, in_=bf[:, :])
            bg_sb = consts.tile([CH, 1], F32)
            nc.sync.dma_start(out=bg_sb, in_=bg[:, :])
            gamma_sb = consts.tile([1, 1], F32)
            nc.sync.dma_start(out=gamma_sb, in_=gamma[:, :])
            bh_bcast = consts.tile([C, C], F32)
            bh_ap = bh[:, :]
            nc.sync.dma_start(
                out=bh_bcast,
                in_=bass.AP(
                    tensor=bh_ap.tensor,
                    offset=bh_ap.offset,
                    ap=[[0, C]] + list(bh_ap.ap)[1:],
                ),
            )
            ones_col_f = consts.tile([C, 1], F32)
            nc.vector.memset(ones_col_f, 1.0)
            ones_col = consts.tile([C, 1], BF16)
            nc.vector.tensor_copy(ones_col, ones_col_f)

            # ---- x: fp32 copy for the residual, fp32r copy for matmuls ----
            x_sb = big.tile([C, N], F32)
            xr_sb = big.tile([C, N], F32R)
            for i in range(NBLK):
                sl = slice(i * NB, (i + 1) * NB)
                nc.sync.dma_start(out=x_sb[:, sl], in_=x[:, sl])
                nc.gpsimd.dma_start(out=xr_sb[:, sl], in_=x[:, sl])

            # ---- f = Wf x + bf, g = Wg x + bg  -> [64, 4096] each ----
            f_dup = big.tile([C, N], F32R)
            g_dup = big.tile([C, N], F32R)
            for i in range(NBLK):
                sl = slice(i * NB, (i + 1) * NB)
                psum_f = pmm.tile([C, NB], F32, tag="mm")
                nc.tensor.matmul(
                    psum_f[0:CH, :], wfT_sb, xr_sb[:, sl], start=True, stop=True
                )
                nc.vector.tensor_scalar_add(f_dup[0:CH, sl], psum_f[0:CH, :], bf_sb)
                psum_g = pmm.tile([C, NB], F32, tag="mm")
                nc.tensor.matmul(
                    psum_g[0:CH, :], wgT_sb, xr_sb[:, sl], start=True, stop=True
                )
                nc.vector.tensor_scalar_add(g_dup[0:CH, sl], psum_g[0:CH, :], bg_sb)

            # replicate f/g into partitions 64-127 so score matmuls can run
            # as two concurrent row-groups of the PE array
            nc.sync.dma_start(out=f_dup[CH:C, :], in_=f_dup[0:CH, :])
            nc.sync.dma_start(out=g_dup[CH:C, :], in_=g_dup[0:CH, :])

            # ---- hT (h^T stored as 32 column-blocks of [128m, 128c]) ----
            hT_sb = big.tile([C, N], BF16)
            for j in range(NMT):
                sl = slice(j * MT, (j + 1) * MT)
                psum_h = pmm.tile([C, MT], F32, tag="mm")
                nc.tensor.matmul(psum_h, xr_sb[:, sl], whT_sb, start=True, stop=True)
                nc.vector.tensor_add(hT_sb[:, sl], psum_h, bh_bcast)

            # ---- main attention loop: 2 n-blocks per round so each
            # stationary operand (f_j / hT_j) serves 2 back-to-back matmuls
            NPAIR = NBLK // 2
            for pr in range(NPAIR):
                nsl2 = slice(pr * 2 * NB, (pr + 1) * 2 * NB)      # both blocks
                nsl_a = slice(pr * 2 * NB, pr * 2 * NB + NB)
                nsl_b = slice(pr * 2 * NB + NB, (pr + 1) * 2 * NB)
                d_acc = dwork.tile([C, 2 * NB], F32, tag="dacc")
                d_acc2 = dwork.tile([C, 2 * NB], F32, tag="dacc2")
                psum_o = po.tile([C, 2 * NB], F32, tag="o")
                psum_d = pd.tile([1, 2 * NB], F32, tag="d")
                ve_seen = 0
                gp_seen = 0
                pe_seen = 0
                pending = []

                def consume(q, e2q):
                    nonlocal ve_seen, gp_seen, pe_seen
                    mslq = slice(q * MT, (q + 1) * MT)
                    lane = q % 3
                    if lane == 0 and ve_seen < ve_groups:
                        if ve_seen == 0:
                            nc.vector.tensor_copy(d_acc, e2q)
                        else:
                            nc.vector.tensor_add(d_acc, d_acc, e2q)
                        ve_seen += 1
                    elif lane == 1 and gp_seen < gp_groups:
                        if gp_seen == 0:
                            nc.gpsimd.tensor_copy(d_acc2, e2q)
                        else:
                            nc.gpsimd.tensor_add(d_acc2, d_acc2, e2q)
                        gp_seen += 1
                    else:
                        nc.tensor.matmul(
                            psum_d[:, 0:NB],
                            ones_col,
                            e2q[:, 0:NB],
                            start=(pe_seen == 0),
                            stop=False,
                            skip_group_check=True,
                        )
                        nc.tensor.matmul(
                            psum_d[:, NB : 2 * NB],
                            ones_col,
                            e2q[:, NB : 2 * NB],
                            start=(pe_seen == 0),
                            stop=False,
                            skip_group_check=True,
                        )
                        pe_seen += 1
                    nc.tensor.matmul(
                        psum_o[:, 0:NB],
                        hT_sb[:, mslq],
                        e2q[:, 0:NB],
                        start=(q == 0),
                        stop=False,
                        skip_group_check=True,
                    )
                    nc.tensor.matmul(
                        psum_o[:, NB : 2 * NB],
                        hT_sb[:, mslq],
                        e2q[:, NB : 2 * NB],
                        start=(q == 0),
                        stop=(q == NMT - 1),
                        skip_group_check=True,
                    )

                for j0 in range(0, NMT, 2):
                    # two score groups back-to-back: one fp32r stream, then
                    # one bf16 consume stream (fewer PE mode switches)
                    for j in (j0, j0 + 1):
                        msl = slice(j * MT, (j + 1) * MT)
                        t2 = pmm.tile([C, 2 * NB], F32, tag="mm")
                        nc.tensor.matmul(
                            t2[:, 0:NB],
                            f_dup[0:CH, msl],
                            g_dup[0:CH, nsl_a],
                            start=True,
                            stop=True,
                        )
                        nc.tensor.matmul(
                            t2[:, NB : 2 * NB],
                            f_dup[CH:C, msl],
                            g_dup[CH:C, nsl_b],
                            start=True,
                            stop=True,
                        )
                        e2 = ework.tile([C, 2 * NB], BF16, tag="e")
                        nc.scalar.activation(e2, t2, mybir.ActivationFunctionType.Exp)
                        pending.append((j, e2))
                    while len(pending) > 2:
                        consume(*pending.pop(0))
                while pending:
                    consume(*pending.pop(0))
                # merge gpsimd partial, then fold both halves (fp32 LOW_HIGH)
                nc.vector.tensor_add(d_acc, d_acc, d_acc2)
                nc.tensor.matmul(
                    psum_d[:, 0:NB],
                    ones_col_f,
                    d_acc[:, 0:NB],
                    start=(pe_seen == 0),
                    stop=False,
                    skip_group_check=True,
                )
                nc.tensor.matmul(
                    psum_d[:, NB : 2 * NB],
                    ones_col_f,
                    d_acc[:, NB : 2 * NB],
                    start=(pe_seen == 0),
                    stop=True,
                    skip_group_check=True,
                )
                d_f32 = small.tile([1, 2 * NB], F32, tag="drec")
                nc.vector.reciprocal(d_f32, psum_d)
                d_g = small.tile([1, 2 * NB], F32, tag="dg")
                nc.vector.tensor_scalar_mul(d_g, d_f32, gamma_sb)
                # broadcast gamma/d across partitions via a DRAM bounce
                nc.sync.dma_start(out=dscratch[2 * pr : 2 * pr + 2, :], in_=d_g)
                b_sb = small.tile([C, 2 * NB], F32, tag="bsb")
                dsc = dscratch[2 * pr : 2 * pr + 2, :]
                dsc_flat = bass.AP(
                    tensor=dsc.tensor,
                    offset=dsc.offset,
                    ap=[[0, C], [1, 2 * NB]],
                )
                nc.sync.dma_start(out=b_sb, in_=dsc_flat)
                res = small.tile([C, 2 * NB], F32, tag="res")
                # evict psum_o as soon as accumulation stops so the next
                # round's o-matmuls aren't blocked on the d/recip chain
                nc.vector.tensor_copy(res, psum_o)
                nc.vector.tensor_mul(res, res, b_sb)
                nc.vector.tensor_add(res, res, x_sb[:, nsl2])
                nc.sync.dma_start(out=out[:, nsl2], in_=res)

    return nc


_NC = None


def get_nc() -> bass.Bass:
    global _NC
    if _NC is None:
        _NC = build_nc()
    return _NC


def make_in_maps(inputs: dict) -> list[dict]:
    x = np.ascontiguousarray(np.asarray(inputs["x"], dtype=np.float32))
    Wf = np.asarray(inputs["Wf"], dtype=np.float32)
    Wg = np.asarray(inputs["Wg"], dtype=np.float32)
    Wh = np.asarray(inputs["Wh"], dtype=np.float32)
    bf = np.asarray(inputs["bf"], dtype=np.float32)
    bg = np.asarray(inputs["bg"], dtype=np.float32)
    bh = np.asarray(inputs["bh"], dtype=np.float32)
    gamma = np.asarray(inputs["gamma"], dtype=np.float32)

    wfT = np.ascontiguousarray(Wf.T)                                  # [128,64]
    wgT = np.ascontiguousarray(Wg.T)                                  # [128,64]
    whT = np.ascontiguousarray(Wh.T)                                  # [128,128]
    bf_c = np.ascontiguousarray(bf[:, None])                          # [64,1]
    bg_c = np.ascontiguousarray(bg[:, None])                          # [64,1]
    bh_row = np.ascontiguousarray(bh[None, :])                        # [1,128]
    gam = np.ascontiguousarray(gamma.reshape(1, 1))                   # [1,1]

    in_maps = []
    for b in range(B):
        in_maps.append(
            {
                "x": np.ascontiguousarray(x[b].reshape(C, N)),
                "wfT": wfT,
                "wgT": wgT,
                "bf": bf_c,
                "bg": bg_c,
                "whT": whT,
                "bh": bh_row,
                "gamma": gam,
            }
        )
    return in_maps


def kernel(**inputs) -> np.ndarray:
    nc = get_nc()
    in_maps = make_in_maps(inputs)
    res = run_bass_kernel_spmd(nc, in_maps, core_ids=list(range(B)))
    out = np.stack([res.results[b]["out"].reshape(C, HH, WW) for b in range(B)])
    return out.astype(np.float32)


# revision 22
# speedup vs baseline: 1.5045x; 1.0278x over previous
"""SAGAN-style attention (nn_Attention_24927990186686) on 8 TRN2 cores.

reference:
  f = Wf@x+bf  [B,64,N]   g = Wg@x+bg  [B,64,N]   h = Wh@x+bh  [B,128,N]
  s = g^T f    [B,N,N]    beta = softmax(s, -1)
  o[c,n] = sum_m beta[n,m] h[c,m];  out = gamma*o + x     (B=8, N=4096)

Sharding: data-parallel over batch, one batch per core, params replicated.

Per-core algorithm ("orientation B" — score tiles transposed so the softmax
contraction (m) lands on the partition axis, which is what the second matmul
contracts over; no giant transposes needed):
  fg  = [Wf^T|Wg^T]^T @ x + [bf;bg]       [128, 4096]  (f rows 0:64, g 64:128)
  hT_j = (x_tile_j)^T... via matmul(lhsT=x[:,128j:128j+128], rhs=Wh^T) + bh
  per n-block (512 cols):
    for j in 32:  t_j = f_j^T g_blk (PSUM) -> e_j = exp(t_j) (ACT)
                  d_acc += e_j (DVE);  psum_o += hT_j^T e_j (PE, accum)
    d = ones^T d_acc (PE row-reduce) -> recip*gamma (DVE)
    bcast via ones outer-product (PE) -> out = psum_o*bcast + x (DVE)
softmax max-subtraction is skipped: |s| <~ 50 for these input distributions,
exp stays comfortably inside fp32 range, and normalization cancels the shift.
"""

import json
import sys
import types

if "/opt/trn_rl_repo" not in sys.path:
    sys.path.insert(0, "/opt/trn_rl_repo")

import numpy as np

import concourse.bass as bass
import concourse.tile as tile
from concourse import mybir
from concourse.bass_utils import run_bass_kernel_spmd
from concourse.vector_clock import ScopedClock

B, C, HH, WW = 8, 128, 64, 64
N = HH * WW          # 4096
CH = C // 2          # 64
NB = 512             # n-block (one PSUM bank of fp32)
NBLK = N // NB       # 8
MT = 128             # m-tile
NMT = N // MT        # 32
F32 = mybir.dt.float32
F32R = mybir.dt.float32r
BF16 = mybir.dt.bfloat16


def _patched_drain_and_barrier(self, tick_clock, wait_clock):
    # Walrus in this env rejects >1-2 sync waits on the Tile tail Drain
    # ("Too many sync wait commands"). Emit the waits as separate SP
    # instructions, then a bare drain.
    nc = self.nc
    carrier = nc.sync.nop(hint="tail_wait_carrier", nofuse=True)
    wait_clock.add_sem_waits(
        carrier.ins, ScopedClock({None: tick_clock.global_clock})
    )
    waits = list(carrier.ins.sync_info.on_wait)
    carrier.ins.sync_info.on_wait = waits[:1]
    sem_by_name = {h.name: h for h in wait_clock.sems.allocated().values()}
    for w in waits[1:]:
        nc.sync.wait_ge(sem_by_name[w.ant_name], w.wait_value)
    nc.sync.drain()
    nc.all_engine_barrier()
    assert self.sems is not None
    popped = nc._tile_sem_poison_stack.pop()
    assert popped is self._sem_poison
    nc.clear_and_free_semaphores(list(self.sems.allocated().values()))
    nc.all_engine_barrier()


tile.TileContext._drain_and_barrier = _patched_drain_and_barrier


def _split_waits_json(bir_bytes: bytes) -> bytes:
    """Walrus here supports only one sync-wait command per instruction.
    Hoist extra waits onto same-engine NoOps inserted just before."""
    bir = json.loads(bir_bytes)
    for func in bir["functions"]:
        for blk in func["blocks"]:
            new = []
            for ins in blk["instructions"]:
                si = ins.get("sync_info")
                waits = si.get("on_wait", []) if si else []
                if len(waits) > 1:
                    for k, w in enumerate(waits[:-1]):
                        nop = {
                            "engine": ins["engine"],
                            "ins": [],
                            "outs": [],
                            "name": f'{ins["name"]}.w{k}',
                            "opcode": "NoOp",
                            "sync_info": {"on_update": [], "on_wait": [w]},
                            "text_hint": "wait_split",
                        }
                        if ins.get("debug") is not None:
                            nop["debug"] = ins["debug"]
                        new.append(nop)
                    si["on_wait"] = waits[-1:]
                new.append(ins)
            blk["instructions"] = new
    return json.dumps(bir).encode()


def _patched_to_json_bytes(self) -> bytes:
    return _split_waits_json(mybir.module_to_json_bytes(self.m))


def build_nc(ve_groups: int = 10, gp_groups: int = 10) -> bass.Bass:
    """Of the 16 m-tile pairs per n-block, ve_groups accumulate the softmax
    denominator on VectorE, gp_groups on GpSimd, the rest via PE
    ones-matmuls (bf16 rhs, fp32 PSUM accumulation)."""
    nc = bass.Bass(trn_type="TRN2")
    nc.to_json_bytes = types.MethodType(_patched_to_json_bytes, nc)
    x = nc.dram_tensor("x", [C, N], F32, kind="ExternalInput")
    wfT = nc.dram_tensor("wfT", [C, CH], F32, kind="ExternalInput")  # Wf^T
    wgT = nc.dram_tensor("wgT", [C, CH], F32, kind="ExternalInput")  # Wg^T
    bf = nc.dram_tensor("bf", [CH, 1], F32, kind="ExternalInput")
    bg = nc.dram_tensor("bg", [CH, 1], F32, kind="ExternalInput")
    whT = nc.dram_tensor("whT", [C, C], F32, kind="ExternalInput")   # Wh^T
    bh = nc.dram_tensor("bh", [1, C], F32, kind="ExternalInput")
    gamma = nc.dram_tensor("gamma", [1, 1], F32, kind="ExternalInput")
    out = nc.dram_tensor("out", [C, N], F32, kind="ExternalOutput")
    dscratch = nc.dram_tensor("dscratch", [NBLK, NB], F32)
    dscratch2 = nc.dram_tensor("dscratch2", [NBLK, NB], F32)

    NG = NMT // 2  # 16 m-tile pairs ("groups") per n-block

    with tile.TileContext(nc) as tc:
        with (
            tc.tile_pool(name="big", bufs=1) as big,
            tc.tile_pool(name="consts", bufs=1) as consts,
            tc.tile_pool(name="ework", bufs=6) as ework,
            tc.tile_pool(name="dwork", bufs=2) as dwork,
            tc.tile_pool(name="small", bufs=2) as small,
            tc.tile_pool(name="pmm", bufs=2, space="PSUM") as pmm,
            tc.tile_pool(name="po", bufs=1, space="PSUM") as po,
            tc.tile_pool(name="pd", bufs=1, space="PSUM") as pd,
        ):
            # ---- constants / params (matmul operands in fp32r) ----
            wfT_sb = consts.tile([C, CH], F32R)
            nc.gpsimd.dma_start(out=wfT_sb, in_=wfT[:, :])
            wgT_sb = consts.tile([C, CH], F32R)
            nc.gpsimd.dma_start(out=wgT_sb, in_=wgT[:, :])
            whT_sb = consts.tile([C, C], F32R)
            nc.gpsimd.dma_start(out=whT_sb, in_=whT[:, :])
            bf_sb = consts.tile([CH, 1], F32)
            nc.sync.dma_start(out=bf_sb, in_=bf[:, :])
            bg_sb = consts.tile([CH, 1], F32)
            nc.sync.dma_start(out=bg_sb, in_=bg[:, :])
            gamma_sb = consts.tile([1, 1], F32)
            nc.sync.dma_start(out=gamma_sb, in_=gamma[:, :])
            gamma_bc = consts.tile([C, 1], F32)
            g_ap = gamma[:, :]
            nc.sync.dma_start(
                out=gamma_bc,
                in_=bass.AP(
                    tensor=g_ap.tensor,
                    offset=g_ap.offset,
                    ap=[[0, C]] + list(g_ap.ap)[1:],
                ),
            )
            bh_bcast = consts.tile([C, C], F32)
            bh_ap = bh[:, :]
            nc.sync.dma_start(
                out=bh_bcast,
                in_=bass.AP(
                    tensor=bh_ap.tensor,
                    offset=bh_ap.offset,
                    ap=[[0, C]] + list(bh_ap.ap)[1:],
                ),
            )
            ones_col_f = consts.tile([C, 1], F32)
            nc.vector.memset(ones_col_f, 1.0)
            ones_col = consts.tile([C, 1], BF16)
            nc.vector.tensor_copy(ones_col, ones_col_f)

            # ---- x: fp32 copy for the residual, fp32r copy for matmuls ----
            x_sb = big.tile([C, N], F32)
            xr_sb = big.tile([C, N], F32R)
            for i in range(NBLK):
                sl = slice(i * NB, (i + 1) * NB)
                nc.sync.dma_start(out=x_sb[:, sl], in_=x[:, sl])
                nc.gpsimd.dma_start(out=xr_sb[:, sl], in_=x[:, sl])

            # ---- f = Wf x + bf, g = Wg x + bg  -> [64, 4096] each ----
            f_dup = big.tile([C, N], F32R)
            g_dup = big.tile([C, N], F32R)
            for i in range(NBLK):
                sl = slice(i * NB, (i + 1) * NB)
                psum_f = pmm.tile([C, NB], F32, tag="mm")
                nc.tensor.matmul(
                    psum_f[0:CH, :], wfT_sb, xr_sb[:, sl], start=True, stop=True
                )
                nc.vector.tensor_scalar_add(f_dup[0:CH, sl], psum_f[0:CH, :], bf_sb)
                psum_g = pmm.tile([C, NB], F32, tag="mm")
                nc.tensor.matmul(
                    psum_g[0:CH, :], wgT_sb, xr_sb[:, sl], start=True, stop=True
                )
                nc.vector.tensor_scalar_add(g_dup[0:CH, sl], psum_g[0:CH, :], bg_sb)
                # replicate this chunk into partitions 64-127 right away so
                # round 0 of the main loop isn't gated on the whole tile
                nc.sync.dma_start(out=f_dup[CH:C, sl], in_=f_dup[0:CH, sl])
                nc.sync.dma_start(out=g_dup[CH:C, sl], in_=g_dup[0:CH, sl])

            # ---- hT (h^T stored as 32 column-blocks of [128m, 128c]) ----
            hT_sb = big.tile([C, N], BF16)
            for j in range(NMT):
                sl = slice(j * MT, (j + 1) * MT)
                psum_h = pmm.tile([C, MT], F32, tag="mm")
                nc.tensor.matmul(psum_h, xr_sb[:, sl], whT_sb, start=True, stop=True)
                nc.vector.tensor_add(hT_sb[:, sl], psum_h, bh_bcast)

            # ---- main attention loop: 2 n-blocks per round so each
            # stationary operand (f_j / hT_j) serves 2 back-to-back matmuls
            NPAIR = NBLK // 2
            for pr in range(NPAIR):
                nsl2 = slice(pr * 2 * NB, (pr + 1) * 2 * NB)      # both blocks
                nsl_a = slice(pr * 2 * NB, pr * 2 * NB + NB)
                nsl_b = slice(pr * 2 * NB + NB, (pr + 1) * 2 * NB)
                d_acc = dwork.tile([C, 2 * NB], F32, tag="dacc")
                d_acc2 = dwork.tile([C, 2 * NB], F32, tag="dacc2")
                psum_o = po.tile([C, 2 * NB], F32, tag="o")
                psum_d = pd.tile([1, 2 * NB], F32, tag="d")
                ve_seen = 0
                gp_seen = 0
                pe_seen = 0
                pending = []

                def consume(q, e2q):
                    nonlocal ve_seen, gp_seen, pe_seen
                    mslq = slice(q * MT, (q + 1) * MT)
                    lane = q % 3
                    if lane == 0 and ve_seen < ve_groups:
                        if ve_seen == 0:
                            nc.vector.tensor_copy(d_acc, e2q)
                        else:
                            nc.vector.tensor_add(d_acc, d_acc, e2q)
                        ve_seen += 1
                    elif lane == 1 and gp_seen < gp_groups:
                        if gp_seen == 0:
                            nc.gpsimd.tensor_copy(d_acc2, e2q)
                        else:
                            nc.gpsimd.tensor_add(d_acc2, d_acc2, e2q)
                        gp_seen += 1
                    else:
                        nc.tensor.matmul(
                            psum_d[:, 0:NB],
                            ones_col,
                            e2q[:, 0:NB],
                            start=(pe_seen == 0),
                            stop=False,
                            skip_group_check=True,
                        )
                        nc.tensor.matmul(
                            psum_d[:, NB : 2 * NB],
                            ones_col,
                            e2q[:, NB : 2 * NB],
                            start=(pe_seen == 0),
                            stop=False,
                            skip_group_check=True,
                        )
                        pe_seen += 1
                    nc.tensor.matmul(
                        psum_o[:, 0:NB],
                        hT_sb[:, mslq],
                        e2q[:, 0:NB],
                        start=(q == 0),
                        stop=False,
                        skip_group_check=True,
                    )
                    nc.tensor.matmul(
                        psum_o[:, NB : 2 * NB],
                        hT_sb[:, mslq],
                        e2q[:, NB : 2 * NB],
                        start=(q == 0),
                        stop=(q == NMT - 1),
                        skip_group_check=True,
                    )

                for j0 in range(0, NMT, 2):
                    # two score groups back-to-back: one fp32r stream, then
                    # one bf16 consume stream (fewer PE mode switches)
                    for j in (j0, j0 + 1):
                        msl = slice(j * MT, (j + 1) * MT)
                        t2 = pmm.tile([C, 2 * NB], F32, tag="mm")
                        nc.tensor.matmul(
                            t2[:, 0:NB],
                            f_dup[0:CH, msl],
                            g_dup[0:CH, nsl_a],
                            start=True,
                            stop=True,
                        )
                        nc.tensor.matmul(
                            t2[:, NB : 2 * NB],
                            f_dup[CH:C, msl],
                            g_dup[CH:C, nsl_b],
                            start=True,
                            stop=True,
                        )
                        e2 = ework.tile([C, 2 * NB], BF16, tag="e")
                        nc.scalar.activation(e2, t2, mybir.ActivationFunctionType.Exp)
                        pending.append((j, e2))
                    while len(pending) > 2:
                        consume(*pending.pop(0))
                while pending:
                    consume(*pending.pop(0))
                # merge gpsimd partial, then fold both halves (fp32 LOW_HIGH)
                nc.vector.tensor_add(d_acc, d_acc, d_acc2)
                nc.tensor.matmul(
                    psum_d[:, 0:NB],
                    ones_col_f,
                    d_acc[:, 0:NB],
                    start=(pe_seen == 0),
                    stop=False,
                    skip_group_check=True,
                )
                nc.tensor.matmul(
                    psum_d[:, NB : 2 * NB],
                    ones_col_f,
                    d_acc[:, NB : 2 * NB],
                    start=(pe_seen == 0),
                    stop=True,
                    skip_group_check=True,
                )
                # reciprocal with all 128 lanes: bounce d through DRAM as
                # [128, 8], recip+scale there, bounce back broadcast
                d_sb = small.tile([1, 2 * NB], F32, tag="dsb")
                nc.scalar.copy(d_sb, psum_d)
                nc.sync.dma_start(out=dscratch2[2 * pr : 2 * pr + 2, :], in_=d_sb)
                dsc2 = dscratch2[2 * pr : 2 * pr + 2, :]
                d_t = small.tile([C, 2 * NB // C], F32, tag="dt")
                nc.sync.dma_start(
                    out=d_t,
                    in_=bass.AP(
                        tensor=dsc2.tensor,
                        offset=dsc2.offset,
                        ap=[[2 * NB // C, C], [1, 2 * NB // C]],
                    ),
                )
                nc.vector.reciprocal(d_t, d_t)
                nc.vector.tensor_scalar_mul(d_t, d_t, gamma_bc)
                dsc = dscratch[2 * pr : 2 * pr + 2, :]
                nc.sync.dma_start(
                    out=bass.AP(
                        tensor=dsc.tensor,
                        offset=dsc.offset,
                        ap=[[2 * NB // C, C], [1, 2 * NB // C]],
                    ),
                    in_=d_t,
                )
                b_sb = small.tile([C, 2 * NB], F32, tag="bsb")
                dsc_flat = bass.AP(
                    tensor=dsc.tensor,
                    offset=dsc.offset,
                    ap=[[0, C], [1, 2 * NB]],
                )
                nc.sync.dma_start(out=b_sb, in_=dsc_flat)
                res = small.tile([C, 2 * NB], F32, tag="res")
                # evict psum_o as soon as accumulation stops so the next
                # round's o-matmuls aren't blocked on the d/recip chain
                nc.vector.tensor_copy(res, psum_o)
                nc.vector.tensor_mul(res, res, b_sb)
                nc.vector.tensor_add(res, res, x_sb[:, nsl2])
                nc.sync.dma_start(out=out[:, nsl2], in_=res)

    return nc


_NC = None


def get_nc() -> bass.Bass:
    global _NC
    if _NC is None:
        _NC = build_nc()
    return _NC


def make_in_maps(inputs: dict) -> list[dict]:
    x = np.ascontiguousarray(np.asarray(inputs["x"], dtype=np.float32))
    Wf = np.asarray(inputs["Wf"], dtype=np.float32)
    Wg = np.asarray(inputs["Wg"], dtype=np.float32)
    Wh = np.asarray(inputs["Wh"], dtype=np.float32)
    bf = np.asarray(inputs["bf"], dtype=np.float32)
    bg = np.asarray(inputs["bg"], dtype=np.float32)
    bh = np.asarray(inputs["bh"], dtype=np.float32)
    gamma = np.asarray(inputs["gamma"], dtype=np.float32)

    wfT = np.ascontiguousarray(Wf.T)                                  # [128,64]
    wgT = np.ascontiguousarray(Wg.T)                                  # [128,64]
    whT = np.ascontiguousarray(Wh.T)                                  # [128,128]
    bf_c = np.ascontiguousarray(bf[:, None])                          # [64,1]
    bg_c = np.ascontiguousarray(bg[:, None])                          # [64,1]
    bh_row = np.ascontiguousarray(bh[None, :])                        # [1,128]
    gam = np.ascontiguousarray(gamma.reshape(1, 1))                   # [1,1]

    in_maps = []
    for b in range(B):
        in_maps.append(
            {
                "x": np.ascontiguousarray(x[b].reshape(C, N)),
                "wfT": wfT,
                "wgT": wgT,
                "bf": bf_c,
                "bg": bg_c,
                "whT": whT,
                "bh": bh_row,
                "gamma": gam,
            }
        )
    return in_maps


def kernel(**inputs) -> np.ndarray:
    nc = get_nc()
    in_maps = make_in_maps(inputs)
    res = run_bass_kernel_spmd(nc, in_maps, core_ids=list(range(B)))
    out = np.stack([res.results[b]["out"].reshape(C, HH, WW) for b in range(B)])
    return out.astype(np.float32)


# revision 23
# speedup vs baseline: 1.5445x; 1.0266x over previous
"""SAGAN-style attention (nn_Attention_24927990186686) on 8 TRN2 cores.

reference:
  f = Wf@x+bf  [B,64,N]   g = Wg@x+bg  [B,64,N]   h = Wh@x+bh  [B,128,N]
  s = g^T f    [B,N,N]    beta = softmax(s, -1)
  o[c,n] = sum_m beta[n,m] h[c,m];  out = gamma*o + x     (B=8, N=4096)

Sharding: data-parallel over batch, one batch per core, params replicated.

Per-core algorithm ("orientation B" — score tiles transposed so the softmax
contraction (m) lands on the partition axis, which is what the second matmul
contracts over; no giant transposes needed):
  fg  = [Wf^T|Wg^T]^T @ x + [bf;bg]       [128, 4096]  (f rows 0:64, g 64:128)
  hT_j = (x_tile_j)^T... via matmul(lhsT=x[:,128j:128j+128], rhs=Wh^T) + bh
  per n-block (512 cols):
    for j in 32:  t_j = f_j^T g_blk (PSUM) -> e_j = exp(t_j) (ACT)
                  d_acc += e_j (DVE);  psum_o += hT_j^T e_j (PE, accum)
    d = ones^T d_acc (PE row-reduce) -> recip*gamma (DVE)
    bcast via ones outer-product (PE) -> out = psum_o*bcast + x (DVE)
softmax max-subtraction is skipped: |s| <~ 50 for these input distributions,
exp stays comfortably inside fp32 range, and normalization cancels the shift.
"""

import json
import sys
import types

if "/opt/trn_rl_repo" not in sys.path:
    sys.path.insert(0, "/opt/trn_rl_repo")

import numpy as np

import concourse.bass as bass
import concourse.tile as tile
from concourse import mybir
from concourse.bass_utils import run_bass_kernel_spmd
from concourse.vector_clock import ScopedClock

B, C, HH, WW = 8, 128, 64, 64
N = HH * WW          # 4096
CH = C // 2          # 64
NB = 512             # n-block (one PSUM bank of fp32)
NBLK = N // NB       # 8
MT = 128             # m-tile
NMT = N // MT        # 32
F32 = mybir.dt.float32
F32R = mybir.dt.float32r
BF16 = mybir.dt.bfloat16


def _patched_drain_and_barrier(self, tick_clock, wait_clock):
    # Walrus in this env rejects >1-2 sync waits on the Tile tail Drain
    # ("Too many sync wait commands"). Emit the waits as separate SP
    # instructions, then a bare drain.
    nc = self.nc
    carrier = nc.sync.nop(hint="tail_wait_carrier", nofuse=True)
    wait_clock.add_sem_waits(
        carrier.ins, ScopedClock({None: tick_clock.global_clock})
    )
    waits = list(carrier.ins.sync_info.on_wait)
    carrier.ins.sync_info.on_wait = waits[:1]
    sem_by_name = {h.name: h for h in wait_clock.sems.allocated().values()}
    for w in waits[1:]:
        nc.sync.wait_ge(sem_by_name[w.ant_name], w.wait_value)
    nc.sync.drain()
    nc.all_engine_barrier()
    assert self.sems is not None
    popped = nc._tile_sem_poison_stack.pop()
    assert popped is self._sem_poison
    nc.clear_and_free_semaphores(list(self.sems.allocated().values()))
    nc.all_engine_barrier()


tile.TileContext._drain_and_barrier = _patched_drain_and_barrier


def _split_waits_json(bir_bytes: bytes) -> bytes:
    """Walrus here supports only one sync-wait command per instruction.
    Hoist extra waits onto same-engine NoOps inserted just before."""
    bir = json.loads(bir_bytes)
    for func in bir["functions"]:
        for blk in func["blocks"]:
            new = []
            for ins in blk["instructions"]:
                si = ins.get("sync_info")
                waits = si.get("on_wait", []) if si else []
                if len(waits) > 1:
                    for k, w in enumerate(waits[:-1]):
                        nop = {
                            "engine": ins["engine"],
                            "ins": [],
                            "outs": [],
                            "name": f'{ins["name"]}.w{k}',
                            "opcode": "NoOp",
                            "sync_info": {"on_update": [], "on_wait": [w]},
                            "text_hint": "wait_split",
                        }
                        if ins.get("debug") is not None:
                            nop["debug"] = ins["debug"]
                        new.append(nop)
                    si["on_wait"] = waits[-1:]
                new.append(ins)
            blk["instructions"] = new
    return json.dumps(bir).encode()


def _patched_to_json_bytes(self) -> bytes:
    return _split_waits_json(mybir.module_to_json_bytes(self.m))


def _make_lanes(ve: int, gp: int, n: int = 32):
    lanes = []
    cv = cg = cp = 0
    pe = n - ve - gp
    for q in range(n):
        # pick the lane furthest behind its quota
        scores = [
            (cv / ve if ve else 9e9, 0),
            (cg / gp if gp else 9e9, 1),
            (cp / pe if pe else 9e9, 2),
        ]
        lane = min(scores)[1]
        lanes.append(lane)
        if lane == 0:
            cv += 1
        elif lane == 1:
            cg += 1
        else:
            cp += 1
    return lanes


def build_nc(ve_groups: int = 14, gp_groups: int = 6) -> bass.Bass:
    """Of the 16 m-tile pairs per n-block, ve_groups accumulate the softmax
    denominator on VectorE, gp_groups on GpSimd, the rest via PE
    ones-matmuls (bf16 rhs, fp32 PSUM accumulation)."""
    nc = bass.Bass(trn_type="TRN2")
    nc.to_json_bytes = types.MethodType(_patched_to_json_bytes, nc)
    x = nc.dram_tensor("x", [C, N], F32, kind="ExternalInput")
    wfT = nc.dram_tensor("wfT", [C, CH], F32, kind="ExternalInput")  # Wf^T
    wgT = nc.dram_tensor("wgT", [C, CH], F32, kind="ExternalInput")  # Wg^T
    bf = nc.dram_tensor("bf", [CH, 1], F32, kind="ExternalInput")
    bg = nc.dram_tensor("bg", [CH, 1], F32, kind="ExternalInput")
    whT = nc.dram_tensor("whT", [C, C], F32, kind="ExternalInput")   # Wh^T
    bh = nc.dram_tensor("bh", [1, C], F32, kind="ExternalInput")
    gamma = nc.dram_tensor("gamma", [1, 1], F32, kind="ExternalInput")
    out = nc.dram_tensor("out", [C, N], F32, kind="ExternalOutput")
    dscratch = nc.dram_tensor("dscratch", [NBLK, NB], F32)
    dscratch2 = nc.dram_tensor("dscratch2", [NBLK, NB], F32)

    NG = NMT // 2  # 16 m-tile pairs ("groups") per n-block
    _D_LANES = _make_lanes(ve_groups, gp_groups, NMT)

    with tile.TileContext(nc) as tc:
        with (
            tc.tile_pool(name="big", bufs=1) as big,
            tc.tile_pool(name="consts", bufs=1) as consts,
            tc.tile_pool(name="ework", bufs=6) as ework,
            tc.tile_pool(name="dwork", bufs=2) as dwork,
            tc.tile_pool(name="small", bufs=2) as small,
            tc.tile_pool(name="pmm", bufs=2, space="PSUM") as pmm,
            tc.tile_pool(name="po", bufs=1, space="PSUM") as po,
            tc.tile_pool(name="pd", bufs=1, space="PSUM") as pd,
        ):
            # ---- constants / params (matmul operands in fp32r) ----
            wfT_sb = consts.tile([C, CH], F32R)
            nc.gpsimd.dma_start(out=wfT_sb, in_=wfT[:, :])
            wgT_sb = consts.tile([C, CH], F32R)
            nc.gpsimd.dma_start(out=wgT_sb, in_=wgT[:, :])
            whT_sb = consts.tile([C, C], BF16)
            nc.gpsimd.dma_start(out=whT_sb, in_=whT[:, :])
            bf_sb = consts.tile([CH, 1], F32)
            nc.sync.dma_start(out=bf_sb, in_=bf[:, :])
            bg_sb = consts.tile([CH, 1], F32)
            nc.sync.dma_start(out=bg_sb, in_=bg[:, :])
            gamma_sb = consts.tile([1, 1], F32)
            nc.sync.dma_start(out=gamma_sb, in_=gamma[:, :])
            gamma_bc = consts.tile([C, 1], F32)
            g_ap = gamma[:, :]
            nc.sync.dma_start(
                out=gamma_bc,
                in_=bass.AP(
                    tensor=g_ap.tensor,
                    offset=g_ap.offset,
                    ap=[[0, C]] + list(g_ap.ap)[1:],
                ),
            )
            bh_bcast = consts.tile([C, C], F32)
            bh_ap = bh[:, :]
            nc.sync.dma_start(
                out=bh_bcast,
                in_=bass.AP(
                    tensor=bh_ap.tensor,
                    offset=bh_ap.offset,
                    ap=[[0, C]] + list(bh_ap.ap)[1:],
                ),
            )
            ones_col_f = consts.tile([C, 1], F32)
            nc.vector.memset(ones_col_f, 1.0)
            ones_col = consts.tile([C, 1], BF16)
            nc.vector.tensor_copy(ones_col, ones_col_f)

            # ---- x: fp32 copy for the residual, fp32r copy for matmuls ----
            x_sb = big.tile([C, N], F32)
            xr_sb = big.tile([C, N], F32R)
            xb_sb = big.tile([C, N], BF16)
            for i in range(NBLK):
                sl = slice(i * NB, (i + 1) * NB)
                nc.sync.dma_start(out=x_sb[:, sl], in_=x[:, sl])
                nc.gpsimd.dma_start(out=xr_sb[:, sl], in_=x[:, sl])
                nc.gpsimd.dma_start(out=xb_sb[:, sl], in_=x[:, sl])

            # ---- f = Wf x + bf, g = Wg x + bg  -> [64, 4096] each ----
            f_dup = big.tile([C, N], F32R)
            g_dup = big.tile([C, N], F32R)
            hT_sb = big.tile([C, N], BF16)
            for i in range(NBLK):
                sl = slice(i * NB, (i + 1) * NB)
                psum_f = pmm.tile([C, NB], F32, tag="mm")
                nc.tensor.matmul(
                    psum_f[0:CH, :], wfT_sb, xr_sb[:, sl], start=True, stop=True
                )
                nc.vector.tensor_scalar_add(f_dup[0:CH, sl], psum_f[0:CH, :], bf_sb)
                psum_g = pmm.tile([C, NB], F32, tag="mm")
                nc.tensor.matmul(
                    psum_g[0:CH, :], wgT_sb, xr_sb[:, sl], start=True, stop=True
                )
                nc.vector.tensor_scalar_add(g_dup[0:CH, sl], psum_g[0:CH, :], bg_sb)
                # replicate this chunk into partitions 64-127 right away so
                # round 0 of the main loop isn't gated on the whole tile
                nc.sync.dma_start(out=f_dup[CH:C, sl], in_=f_dup[0:CH, sl])
                nc.sync.dma_start(out=g_dup[CH:C, sl], in_=g_dup[0:CH, sl])
                # hT for this chunk's 4 m-tiles (bf16 path)
                for j in range(4 * i, 4 * i + 4):
                    slj = slice(j * MT, (j + 1) * MT)
                    psum_h = pmm.tile([C, MT], F32, tag="mm")
                    nc.tensor.matmul(
                        psum_h, xb_sb[:, slj], whT_sb, start=True, stop=True
                    )
                    nc.vector.tensor_add(hT_sb[:, slj], psum_h, bh_bcast)

            # ---- main attention loop: 2 n-blocks per round so each
            # stationary operand (f_j / hT_j) serves 2 back-to-back matmuls
            NPAIR = NBLK // 2
            for pr in range(NPAIR):
                nsl2 = slice(pr * 2 * NB, (pr + 1) * 2 * NB)      # both blocks
                nsl_a = slice(pr * 2 * NB, pr * 2 * NB + NB)
                nsl_b = slice(pr * 2 * NB + NB, (pr + 1) * 2 * NB)
                d_acc = dwork.tile([C, 2 * NB], F32, tag="dacc")
                d_acc2 = dwork.tile([C, 2 * NB], F32, tag="dacc2")
                psum_o = po.tile([C, 2 * NB], F32, tag="o")
                psum_d = pd.tile([1, 2 * NB], F32, tag="d")
                ve_seen = 0
                gp_seen = 0
                pe_seen = 0
                pending = []

                def consume(q, e2q):
                    nonlocal ve_seen, gp_seen, pe_seen
                    mslq = slice(q * MT, (q + 1) * MT)
                    lane = _D_LANES[q]
                    if lane == 0 and ve_seen < ve_groups:
                        if ve_seen == 0:
                            nc.vector.tensor_copy(d_acc, e2q)
                        else:
                            nc.vector.tensor_add(d_acc, d_acc, e2q)
                        ve_seen += 1
                    elif lane == 1 and gp_seen < gp_groups:
                        if gp_seen == 0:
                            nc.gpsimd.tensor_copy(d_acc2, e2q)
                        else:
                            nc.gpsimd.tensor_add(d_acc2, d_acc2, e2q)
                        gp_seen += 1
                    else:
                        nc.tensor.matmul(
                            psum_d[:, 0:NB],
                            ones_col,
                            e2q[:, 0:NB],
                            start=(pe_seen == 0),
                            stop=False,
                            skip_group_check=True,
                        )
                        nc.tensor.matmul(
                            psum_d[:, NB : 2 * NB],
                            ones_col,
                            e2q[:, NB : 2 * NB],
                            start=(pe_seen == 0),
                            stop=False,
                            skip_group_check=True,
                        )
                        pe_seen += 1
                    nc.tensor.matmul(
                        psum_o[:, 0:NB],
                        hT_sb[:, mslq],
                        e2q[:, 0:NB],
                        start=(q == 0),
                        stop=False,
                        skip_group_check=True,
                    )
                    nc.tensor.matmul(
                        psum_o[:, NB : 2 * NB],
                        hT_sb[:, mslq],
                        e2q[:, NB : 2 * NB],
                        start=(q == 0),
                        stop=(q == NMT - 1),
                        skip_group_check=True,
                    )

                for j0 in range(0, NMT, 2):
                    # two score groups back-to-back: one fp32r stream, then
                    # one bf16 consume stream (fewer PE mode switches)
                    for j in (j0, j0 + 1):
                        msl = slice(j * MT, (j + 1) * MT)
                        t2 = pmm.tile([C, 2 * NB], F32, tag="mm")
                        nc.tensor.matmul(
                            t2[:, 0:NB],
                            f_dup[0:CH, msl],
                            g_dup[0:CH, nsl_a],
                            start=True,
                            stop=True,
                        )
                        nc.tensor.matmul(
                            t2[:, NB : 2 * NB],
                            f_dup[CH:C, msl],
                            g_dup[CH:C, nsl_b],
                            start=True,
                            stop=True,
                        )
                        e2 = ework.tile([C, 2 * NB], BF16, tag="e")
                        nc.scalar.activation(e2, t2, mybir.ActivationFunctionType.Exp)
                        pending.append((j, e2))
                    while len(pending) > 2:
                        consume(*pending.pop(0))
                while pending:
                    consume(*pending.pop(0))
                # merge gpsimd partial, then fold both halves (fp32 LOW_HIGH)
                nc.vector.tensor_add(d_acc, d_acc, d_acc2)
                nc.tensor.matmul(
                    psum_d[:, 0:NB],
                    ones_col_f,
                    d_acc[:, 0:NB],
                    start=(pe_seen == 0),
                    stop=False,
                    skip_group_check=True,
                )
                nc.tensor.matmul(
                    psum_d[:, NB : 2 * NB],
                    ones_col_f,
                    d_acc[:, NB : 2 * NB],
                    start=(pe_seen == 0),
                    stop=True,
                    skip_group_check=True,
                )
                # reciprocal with all 128 lanes: bounce d through DRAM as
                # [128, 8], recip+scale there, bounce back broadcast
                d_sb = small.tile([1, 2 * NB], F32, tag="dsb")
                nc.scalar.copy(d_sb, psum_d)
                nc.sync.dma_start(out=dscratch2[2 * pr : 2 * pr + 2, :], in_=d_sb)
                dsc2 = dscratch2[2 * pr : 2 * pr + 2, :]
                d_t = small.tile([C, 2 * NB // C], F32, tag="dt")
                nc.sync.dma_start(
                    out=d_t,
                    in_=bass.AP(
                        tensor=dsc2.tensor,
                        offset=dsc2.offset,
                        ap=[[2 * NB // C, C], [1, 2 * NB // C]],
                    ),
                )
                nc.vector.reciprocal(d_t, d_t)
                nc.vector.tensor_scalar_mul(d_t, d_t, gamma_bc)
                dsc = dscratch[2 * pr : 2 * pr + 2, :]
                nc.sync.dma_start(
                    out=bass.AP(
                        tensor=dsc.tensor,
                        offset=dsc.offset,
                        ap=[[2 * NB // C, C], [1, 2 * NB // C]],
                    ),
                    in_=d_t,
                )
                b_sb = small.tile([C, 2 * NB], F32, tag="bsb")
                dsc_flat = bass.AP(
                    tensor=dsc.tensor,
                    offset=dsc.offset,
                    ap=[[0, C], [1, 2 * NB]],
                )
                nc.sync.dma_start(out=b_sb, in_=dsc_flat)
                res = small.tile([C, 2 * NB], F32, tag="res")
                # evict psum_o as soon as accumulation stops so the next
                # round's o-matmuls aren't blocked on the d/recip chain
                nc.vector.tensor_copy(res, psum_o)
                nc.vector.tensor_mul(res, res, b_sb)
                nc.vector.tensor_add(res, res, x_sb[:, nsl2])
                nc.sync.dma_start(out=out[:, nsl2], in_=res)

    return nc


_NC = None


def get_nc() -> bass.Bass:
    global _NC
    if _NC is None:
        _NC = build_nc()
    return _NC


def make_in_maps(inputs: dict) -> list[dict]:
    x = np.ascontiguousarray(np.asarray(inputs["x"], dtype=np.float32))
    Wf = np.asarray(inputs["Wf"], dtype=np.float32)
    Wg = np.asarray(inputs["Wg"], dtype=np.float32)
    Wh = np.asarray(inputs["Wh"], dtype=np.float32)
    bf = np.asarray(inputs["bf"], dtype=np.float32)
    bg = np.asarray(inputs["bg"], dtype=np.float32)
    bh = np.asarray(inputs["bh"], dtype=np.float32)
    gamma = np.asarray(inputs["gamma"], dtype=np.float32)

    wfT = np.ascontiguousarray(Wf.T)                                  # [128,64]
    wgT = np.ascontiguousarray(Wg.T)                                  # [128,64]
    whT = np.ascontiguousarray(Wh.T)                                  # [128,128]
    bf_c = np.ascontiguousarray(bf[:, None])                          # [64,1]
    bg_c = np.ascontiguousarray(bg[:, None])                          # [64,1]
    bh_row = np.ascontiguousarray(bh[None, :])                        # [1,128]
    gam = np.ascontiguousarray(gamma.reshape(1, 1))                   # [1,1]

    in_maps = []
    for b in range(B):
        in_maps.append(
            {
                "x": np.ascontiguousarray(x[b].reshape(C, N)),
                "wfT": wfT,
                "wgT": wgT,
                "bf": bf_c,
                "bg": bg_c,
                "whT": whT,
                "bh": bh_row,
                "gamma": gam,
            }
        )
    return in_maps


def kernel(**inputs) -> np.ndarray:
    nc = get_nc()
    in_maps = make_in_maps(inputs)
    res = run_bass_kernel_spmd(nc, in_maps, core_ids=list(range(B)))
    out = np.stack([res.results[b]["out"].reshape(C, HH, WW) for b in range(B)])
    return out.astype(np.float32)
